# revision 1
# baseline (speedup 1.0000x reference)
"""Trainium2 Bass kernel v2 for the 2-layer GCN (nn_CGNN_70566312673786).

8 NeuronCores, SPMD, 3 launches, host concat between launches.

launch1: deg via exact scan-prefix extraction, dis, xs=dis*x (bf16 table).
launch2: L1 spmm (one-hot matmul scatter, batched one-hot builds) + W1+relu
         -> hs table (bf16, dis-prescaled for L2's source side).
launch3: L2 spmm + W2 + b2 -> output.

Edge schedule (uniform across cores => one NEFF per launch):
  per (block, bucket): TBB[b,k] = max over cores of ceil(edges/128) tiles.
  Tiles grouped by (block-group of BG blocks, bucket) into segments;
  gather calls chunk each segment (<= CT tiles per call); pads use idx 0
  with weight 0.  One-hot builds are batched: one tensor_tensor(is_equal)
  + one tensor_tensor(mult) over [128, 128*K] in an (r, t) layout whose
  innermost dim is packed (DVE 2x eligible), with stride-0 broadcast rl/cw.
  Accumulation: acc[feat, row] += g_tile.T @ oh_tile per tile (PSUM bank
  per block slot, 7 slots).
"""

import numpy as np
import ml_dtypes

import concourse.bacc as bacc
import concourse.mybir as mybir
import concourse.tile as tile

N = 100000
E = 1600000
D = 128
H = 128
C_OUT = 40
C_PAD = 64

F32 = mybir.dt.float32
BF16 = mybir.dt.bfloat16
I16 = mybir.dt.int16

NCORES = 8
RPC = 12544
NPAD = NCORES * RPC
NBLK = RPC // 128            # 98
NBUCK = 4
BUCK = NPAD // NBUCK         # 25088
BG = 7                       # blocks per PSUM group (7 acc banks + 1 tail)
NGRP = (NBLK + BG - 1) // BG
CT = 32                      # max tiles per dma_gather call / oh batch
SENT = 200.0

BF = ml_dtypes.bfloat16

IDENT = np.eye(128, dtype=np.float32)
ISHIFT = np.eye(128, k=-1, dtype=np.float32).T.copy()   # [p,r]=1 iff p==r-1
IOTA_P = np.tile(np.arange(128, dtype=np.float32)[:, None], (1, 128))


def _wrap_idx(flat):
    n = flat.shape[0]
    w = flat.reshape(n // 16, 16).T
    return np.tile(w, (8, 1))


# ---------------------------------------------------------------------------
# host-side schedule (index prep only)
# ---------------------------------------------------------------------------

def build_layouts(row, col, cv):
    row = np.asarray(row, np.int64)
    col = np.asarray(col, np.int64)
    cv = np.asarray(cv, np.float32)
    core_of = row // RPC

    per_core = []
    for c in range(NCORES):
        m = core_of == c
        per_core.append(((row[m] - c * RPC).astype(np.int64), col[m],
                         cv[m].astype(np.float32)))

    # ---- launch1 arrays ----
    T1 = 1
    cnts = []
    for r, _, _ in per_core:
        cb = np.bincount(r >> 7, minlength=NBLK)
        cnts.append(cb)
        T1 = max(T1, int(np.ceil(cb.max() / 128)))
    l1 = []
    for c in range(NCORES):
        r, _, w = per_core[c]
        o = np.argsort(r, kind="stable")
        rs, ws = r[o], w[o]
        blk = rs >> 7
        start_b = np.zeros(NBLK + 1, np.int64)
        np.cumsum(cnts[c], out=start_b[1:])
        e_in_blk = np.arange(len(rs)) - start_b[blk]
        wv = np.zeros((NBLK, 128 * T1), np.float32)
        wv[blk, e_in_blk] = ws
        # chunk layout [b, p, t]: flat position e -> (p=e//T1, t=e%T1)
        wvs = np.ascontiguousarray(
            wv.reshape(NBLK, 128, T1).transpose(1, 0, 2).reshape(
                128, NBLK * T1))
        ends = np.searchsorted(rs, np.arange(RPC), side="right")
        e_r = ends - start_b[np.arange(RPC) >> 7]
        pos = e_r - 1
        pt = np.where(pos >= 0, pos // T1, SENT).astype(np.float32)
        tt = np.where(pos >= 0, pos % T1, 0).astype(np.float32)
        l1.append({
            "wvs": wvs,
            "ptT": np.ascontiguousarray(pt.reshape(NBLK, 128)),
            "ttB": np.ascontiguousarray(tt.reshape(NBLK, 128).T),
        })

    # ---- launch2/3 uniform tile structure ----
    nbk_cnt = np.zeros((NCORES, NBLK, NBUCK), np.int64)
    for c in range(NCORES):
        r, cc, _ = per_core[c]
        np.add.at(nbk_cnt[c], ((r >> 7), cc // BUCK), 1)
    tbb = np.maximum(1, (nbk_cnt.max(axis=0) + 127) // 128)   # [NBLK, NBUCK]

    # tile order: for grp, for bucket, for block in grp: tbb[b,k] tiles
    tile_blk = []
    tile_bkt = []
    seg_of_call = []
    calls = []          # (bucket, tile0, ct)
    grp_last_call = []  # index of last call per grp
    for g in range(NGRP):
        blks = range(g * BG, min((g + 1) * BG, NBLK))
        for k in range(NBUCK):
            seg_t0 = len(tile_blk)
            for b in blks:
                tile_blk += [b] * int(tbb[b, k])
                tile_bkt += [k] * int(tbb[b, k])
            nt = len(tile_blk) - seg_t0
            t = seg_t0
            while t < seg_t0 + nt:
                ct = min(CT, seg_t0 + nt - t)
                calls.append((k, t, ct))
                t += ct
        grp_last_call.append(len(calls) - 1)
    NT = len(tile_blk)
    tile_blk = np.asarray(tile_blk)
    tile_bkt = np.asarray(tile_bkt)

    # tile offset of each (b,k) run
    run_off = np.zeros((NBLK, NBUCK), np.int64)
    pos = 0
    for g in range(NGRP):
        blks = range(g * BG, min((g + 1) * BG, NBLK))
        for k in range(NBUCK):
            for b in blks:
                run_off[b, k] = pos
                pos += int(tbb[b, k])

    # start/stop tile per block
    first_tile = np.full(NBLK, -1, np.int64)
    last_tile = np.full(NBLK, -1, np.int64)
    for t in range(NT):
        b = tile_blk[t]
        if first_tile[b] < 0:
            first_tile[b] = t
        last_tile[b] = t

    # per-core data arrays
    l2 = []
    for c in range(NCORES):
        r, cc, w = per_core[c]
        bk = cc // BUCK
        o = np.lexsort((r, bk, (r >> 7)))
        r2, c2, w2, k2 = r[o], cc[o], w[o], bk[o]
        b2 = r2 >> 7
        # position within (b,k) run
        key = b2 * NBUCK + k2
        order = np.lexsort((np.arange(len(r2)), key))
        # edges already grouped by (b,k) under lexsort(r, bk, blk)? group by
        # key directly:
        cnt = np.bincount(key, minlength=NBLK * NBUCK)
        st = np.zeros(NBLK * NBUCK + 1, np.int64)
        np.cumsum(cnt, out=st[1:])
        # within-run position for each edge (in 'order' ordering)
        posn = np.empty(len(r2), np.int64)
        posn[order] = np.arange(len(r2)) - st[key[order]]
        slot = run_off[b2, k2] * 128 + posn
        rlt = np.zeros((128, NT), np.float32)
        cwt = np.zeros((128, NT), np.float32)
        idxf = np.zeros(NT * 128, np.int64)
        tile_of = slot >> 7
        rlt[slot & 127, tile_of] = (r2 & 127).astype(np.float32)
        cwt[slot & 127, tile_of] = w2
        idxf[slot] = c2 - k2 * BUCK
        l2.append({"rlt": rlt.astype(BF), "cwt": cwt.astype(BF),
                   "idxw": _wrap_idx(idxf.astype(np.int16))})

    sched = {"NT": NT, "tile_blk": tile_blk, "tile_bkt": tile_bkt,
             "calls": calls, "grp_last_call": grp_last_call,
             "first_tile": first_tile, "last_tile": last_tile}
    return l1, l2, T1, sched


# ---------------------------------------------------------------------------
# launch 1
# ---------------------------------------------------------------------------

def build_launch1(T1, reps=1):
    import contextlib
    nc = bacc.Bacc("TRN2", target_bir_lowering=False)
    x_sl = nc.dram_tensor("x_sl", [RPC, D], F32, kind="ExternalInput")
    wvt = nc.dram_tensor("wvt", [128, NBLK * T1], F32, kind="ExternalInput")
    ptt = nc.dram_tensor("ptt", [NBLK, 128], F32, kind="ExternalInput")
    ttt = nc.dram_tensor("ttt", [128, NBLK], F32, kind="ExternalInput")
    iop = nc.dram_tensor("iop", [128, 128], F32, kind="ExternalInput")
    iof = nc.dram_tensor("iof", [128, T1], F32, kind="ExternalInput")
    isht = nc.dram_tensor("isht", [128, 128], F32, kind="ExternalInput")
    idnt = nc.dram_tensor("idnt", [128, 128], F32, kind="ExternalInput")
    xs_sl = nc.dram_tensor("xs_sl", [RPC, D], BF16, kind="ExternalOutput")
    dis_sl = nc.dram_tensor("dis_sl", [128, NBLK], F32, kind="ExternalOutput")
    disT_sl = nc.dram_tensor("disT_sl", [NBLK, 128], F32,
                             kind="ExternalOutput")

    with tile.TileContext(nc) as tc:
        with tc.tile_pool(name="const", bufs=1) as cpool, \
             tc.tile_pool(name="sc", bufs=1) as scpool, \
             tc.tile_pool(name="work", bufs=4) as wpool, \
             tc.tile_pool(name="small", bufs=4) as spool, \
             tc.tile_pool(name="psA", bufs=2, space="PSUM") as ppool, \
             tc.tile_pool(name="psB", bufs=1, space="PSUM") as ppool2:
            wv = cpool.tile([128, NBLK * T1], F32)
            ttS = cpool.tile([128, NBLK], F32)
            ioP = cpool.tile([128, 128], F32)
            ioF = cpool.tile([128, T1], F32)
            ish = cpool.tile([128, 128], F32)
            idn = cpool.tile([128, 128], F32)
            zT1 = cpool.tile([128, T1], F32)
            znb = cpool.tile([NBLK, 128], F32)
            E_all = cpool.tile([128, NBLK], F32)
            Sall = scpool.tile([128, NBLK * T1], F32)
            totP = cpool.tile([128, NBLK], F32)
            nc.sync.dma_start(out=wv[:], in_=wvt[:, :])
            nc.sync.dma_start(out=ttS[:], in_=ttt[:, :])
            nc.sync.dma_start(out=ioP[:], in_=iop[:, :])
            nc.sync.dma_start(out=ioF[:], in_=iof[:, :])
            nc.sync.dma_start(out=ish[:], in_=isht[:, :])
            nc.sync.dma_start(out=idn[:], in_=idnt[:, :])
            nc.vector.memset(zT1[:], 0.0)
            nc.vector.memset(znb[:], 0.0)

            rep = tc.For_i(0, reps, 1) if reps > 1 else contextlib.nullcontext()
            with rep:
                # A: per-block chunk scans; totals
                for b in range(NBLK):
                    nc.vector.tensor_tensor_scan(
                        out=Sall[:, b * T1:(b + 1) * T1],
                        data0=wv[:, b * T1:(b + 1) * T1],
                        data1=zT1[:], initial=0.0,
                        op0=mybir.AluOpType.add, op1=mybir.AluOpType.add)
                    nc.vector.tensor_copy(
                        out=totP[:, b:b + 1],
                        in_=Sall[:, b * T1 + T1 - 1:b * T1 + T1])
                # carry: transpose -> per-block scan along chunks -> back
                tot_ps = ppool2.tile([NBLK, 128], F32, tag="tps")
                nc.tensor.transpose(out=tot_ps[:], in_=totP[:],
                                    identity=idn[:])
                totT = spool.tile([NBLK, 128], F32, tag="totT")
                nc.vector.tensor_copy(out=totT[:], in_=tot_ps[:])
                incl = spool.tile([NBLK, 128], F32, tag="incl")
                nc.vector.tensor_tensor_scan(
                    out=incl[:], data0=totT[:], data1=znb[:], initial=0.0,
                    op0=mybir.AluOpType.add, op1=mybir.AluOpType.add)
                excl = spool.tile([NBLK, 128], F32, tag="excl")
                nc.vector.tensor_tensor(
                    out=excl[:], in0=incl[:], in1=totT[:],
                    op=mybir.AluOpType.subtract)
                car_ps = ppool2.tile([128, NBLK], F32, tag="cps")
                nc.tensor.transpose(out=car_ps[:], in_=excl[:],
                                    identity=idn[:NBLK, :NBLK])
                carry = spool.tile([128, NBLK], F32, tag="carry")
                nc.vector.tensor_copy(out=carry[:], in_=car_ps[:])

                # B: per-block boundary extraction
                for b in range(NBLK):
                    pref = wpool.tile([128, T1], F32, tag="pref")
                    nc.vector.tensor_scalar(
                        out=pref[:], in0=Sall[:, b * T1:(b + 1) * T1],
                        scalar1=carry[:, b:b + 1], scalar2=None,
                        op0=mybir.AluOpType.add)
                    ptr = wpool.tile([1, 128], F32, tag="ptr")
                    nc.sync.dma_start(out=ptr[:], in_=ptt[b:b + 1, :])
                    ptB = wpool.tile([128, 128], F32, tag="ptB")
                    nc.gpsimd.partition_broadcast(
                        out_ap=ptB[:], in_ap=ptr[:])
                    E1 = wpool.tile([128, 128], F32, tag="E1")
                    nc.vector.tensor_tensor(
                        out=E1[:], in0=ioP[:], in1=ptB[:],
                        op=mybir.AluOpType.is_equal)
                    M_ps = ppool.tile([128, T1], F32, tag="M")
                    nc.tensor.matmul(out=M_ps[:], lhsT=E1[:], rhs=pref[:],
                                     start=True, stop=True)
                    sel = wpool.tile([128, T1], F32, tag="sel")
                    nc.vector.tensor_scalar(
                        out=sel[:], in0=ioF[:], scalar1=ttS[:, b:b + 1],
                        scalar2=None, op0=mybir.AluOpType.is_equal)
                    Msel = wpool.tile([128, T1], F32, tag="Msel")
                    nc.vector.tensor_tensor(
                        out=Msel[:], in0=M_ps[:], in1=sel[:],
                        op=mybir.AluOpType.mult)
                    nc.vector.tensor_reduce(
                        out=E_all[:, b:b + 1], in_=Msel[:],
                        axis=mybir.AxisListType.X, op=mybir.AluOpType.add)

                # C: deg = E - shift(E); dis = rsqrt guard
                Ep_ps = ppool.tile([128, NBLK], F32, tag="Ep")
                nc.tensor.matmul(out=Ep_ps[:], lhsT=ish[:], rhs=E_all[:],
                                 start=True, stop=True)
                deg = spool.tile([128, NBLK], F32, tag="deg")
                nc.vector.tensor_tensor(
                    out=deg[:], in0=E_all[:], in1=Ep_ps[:],
                    op=mybir.AluOpType.subtract)
                z = spool.tile([128, NBLK], F32, tag="z")
                nc.vector.tensor_scalar(
                    out=z[:], in0=deg[:], scalar1=0.0, scalar2=None,
                    op0=mybir.AluOpType.is_le)
                nc.vector.tensor_tensor(
                    out=deg[:], in0=deg[:], in1=z[:], op=mybir.AluOpType.add)
                sq = spool.tile([128, NBLK], F32, tag="sq")
                nc.scalar.sqrt(out=sq[:], in_=deg[:])
                rec = spool.tile([128, NBLK], F32, tag="rec")
                nc.vector.reciprocal(out=rec[:], in_=sq[:])
                nc.vector.tensor_scalar(
                    out=z[:], in0=z[:], scalar1=-1.0, scalar2=1.0,
                    op0=mybir.AluOpType.mult, op1=mybir.AluOpType.add)
                dis = spool.tile([128, NBLK], F32, tag="dis")
                nc.vector.tensor_tensor(
                    out=dis[:], in0=rec[:], in1=z[:], op=mybir.AluOpType.mult)
                nc.sync.dma_start(out=dis_sl[:, :], in_=dis[:])
                dT_ps = ppool2.tile([NBLK, 128], F32, tag="dT")
                nc.tensor.transpose(out=dT_ps[:], in_=dis[:], identity=idn[:])
                dT = spool.tile([NBLK, 128], F32, tag="dTs")
                nc.vector.tensor_copy(out=dT[:], in_=dT_ps[:])
                nc.sync.dma_start(out=disT_sl[:, :], in_=dT[:])

                # D: xs = dis * x -> bf16
                for b in range(NBLK):
                    xt = wpool.tile([128, D], F32, tag="xt")
                    nc.sync.dma_start(out=xt[:],
                                      in_=x_sl[b * 128:(b + 1) * 128, :])
                    xst = wpool.tile([128, D], BF16, tag="xst")
                    nc.vector.tensor_scalar(
                        out=xst[:], in0=xt[:], scalar1=dis[:, b:b + 1],
                        scalar2=None, op0=mybir.AluOpType.mult)
                    nc.sync.dma_start(out=xs_sl[b * 128:(b + 1) * 128, :],
                                      in_=xst[:])
    nc.compile()
    return nc


# ---------------------------------------------------------------------------
# launches 2/3
# ---------------------------------------------------------------------------

def build_spmm(sched, layer, reps=1, ablate=()):
    import contextlib
    NT = sched["NT"]
    calls = sched["calls"]
    tile_blk = sched["tile_blk"]
    first_tile = sched["first_tile"]
    last_tile = sched["last_tile"]
    grp_last_call = set(sched["grp_last_call"])

    nc = bacc.Bacc("TRN2", target_bir_lowering=False, num_swdge_queues=4)
    tab = nc.dram_tensor("tab", [NPAD, D], BF16, kind="ExternalInput")
    rlt = nc.dram_tensor("rlt", [128, NT], BF16, kind="ExternalInput")
    cwt = nc.dram_tensor("cwt", [128, NT], BF16, kind="ExternalInput")
    idxt = nc.dram_tensor("idxt", [128, NT * 8], I16, kind="ExternalInput")
    iot = nc.dram_tensor("iot", [128, 128 * CT], BF16, kind="ExternalInput")
    dis_t = nc.dram_tensor("dis_t", [128, NBLK], F32, kind="ExternalInput")
    if layer == 1:
        disT_t = nc.dram_tensor("disT_t", [NBLK, 128], F32,
                                kind="ExternalInput")
        w1t = nc.dram_tensor("w1t", [D, H], BF16, kind="ExternalInput")
        b1t = nc.dram_tensor("b1t", [H, 1], F32, kind="ExternalInput")
        idnt = nc.dram_tensor("idnt", [128, 128], BF16,
                              kind="ExternalInput")
        out_sl = nc.dram_tensor("out_sl", [RPC, D], BF16,
                                kind="ExternalOutput")
    else:
        w2t = nc.dram_tensor("w2t", [H, C_PAD], BF16, kind="ExternalInput")
        b2t = nc.dram_tensor("b2t", [128, C_PAD], F32, kind="ExternalInput")
        out_sl = nc.dram_tensor("out_sl", [RPC, C_PAD], F32,
                                kind="ExternalOutput")

    with tile.TileContext(nc) as tc:
        with tc.tile_pool(name="const", bufs=1) as cpool, \
             tc.tile_pool(name="gat", bufs=3) as gpool, \
             tc.tile_pool(name="oh", bufs=3) as opool, \
             tc.tile_pool(name="tail", bufs=3) as tpool, \
             tc.tile_pool(name="acc", bufs=1, space="PSUM") as apool, \
             tc.tile_pool(name="pt", bufs=1, space="PSUM") as ppool2:
            rl = cpool.tile([128, NT], BF16)
            cw = cpool.tile([128, NT], BF16)
            idxs = cpool.tile([128, NT * 8], I16)
            io = cpool.tile([128, 128 * CT], BF16)
            dis = cpool.tile([128, NBLK], F32)
            nc.sync.dma_start(out=rl[:], in_=rlt[:, :])
            nc.sync.dma_start(out=cw[:], in_=cwt[:, :])
            nc.sync.dma_start(out=idxs[:], in_=idxt[:, :])
            nc.sync.dma_start(out=io[:], in_=iot[:, :])
            nc.sync.dma_start(out=dis[:], in_=dis_t[:, :])
            if layer == 1:
                w1s = cpool.tile([D, H], BF16)
                b1s = cpool.tile([H, 1], F32)
                idn = cpool.tile([128, 128], BF16)
                nc.sync.dma_start(out=w1s[:], in_=w1t[:, :])
                nc.sync.dma_start(out=b1s[:], in_=b1t[:, :])
                nc.sync.dma_start(out=idn[:], in_=idnt[:, :])
            else:
                w2s = cpool.tile([H, C_PAD], BF16)
                b2s = cpool.tile([128, C_PAD], F32)
                nc.sync.dma_start(out=w2s[:], in_=w2t[:, :])
                nc.sync.dma_start(out=b2s[:], in_=b2t[:, :])

            accs = {}

            def tail(b):
                acc = accs.pop(b)
                if layer == 1:
                    dr = tpool.tile([1, 128], F32, tag="dr")
                    nc.sync.dma_start(out=dr[:], in_=disT_t[b:b + 1, :])
                    dB = tpool.tile([128, 128], F32, tag="dB")
                    nc.gpsimd.partition_broadcast(
                        out_ap=dB[:], in_ap=dr[:])
                    s_sb = tpool.tile([128, 128], BF16, tag="s_sb")
                    nc.vector.tensor_tensor(
                        out=s_sb[:], in0=acc[:], in1=dB[:],
                        op=mybir.AluOpType.mult)
                    hT_ps = ppool2.tile([H, 128], F32, tag="hT")
                    nc.tensor.matmul(out=hT_ps[:], lhsT=w1s[:], rhs=s_sb[:],
                                     start=True, stop=True)
                    ht_sb = tpool.tile([H, 128], BF16, tag="ht_sb")
                    nc.scalar.activation(
                        out=ht_sb[:], in_=hT_ps[:],
                        func=mybir.ActivationFunctionType.Relu,
                        bias=b1s[:, 0:1], scale=1.0)
                    hh_ps = ppool2.tile([128, H], BF16, tag="hT")
                    nc.tensor.transpose(out=hh_ps[:], in_=ht_sb[:],
                                        identity=idn[:])
                    hs_sb = tpool.tile([128, H], BF16, tag="hs_sb")
                    nc.vector.tensor_scalar(
                        out=hs_sb[:], in0=hh_ps[:], scalar1=dis[:, b:b + 1],
                        scalar2=None, op0=mybir.AluOpType.mult)
                    nc.sync.dma_start(out=out_sl[b * 128:(b + 1) * 128, :],
                                      in_=hs_sb[:])
                else:
                    s_sb = tpool.tile([128, 128], BF16, tag="s_sb")
                    nc.vector.tensor_copy(out=s_sb[:], in_=acc[:])
                    o_ps = ppool2.tile([128, C_PAD], F32, tag="hT")
                    nc.tensor.matmul(out=o_ps[:], lhsT=s_sb[:], rhs=w2s[:],
                                     start=True, stop=True)
                    o_sb = tpool.tile([128, C_PAD], F32, tag="o_sb")
                    nc.vector.tensor_scalar(
                        out=o_sb[:], in0=o_ps[:], scalar1=dis[:, b:b + 1],
                        scalar2=None, op0=mybir.AluOpType.mult)
                    nc.vector.tensor_tensor(
                        out=o_sb[:], in0=o_sb[:], in1=b2s[:],
                        op=mybir.AluOpType.add)
                    nc.sync.dma_start(out=out_sl[b * 128:(b + 1) * 128, :],
                                      in_=o_sb[:])

            rep = tc.For_i(0, reps, 1) if reps > 1 else contextlib.nullcontext()
            with rep:
                for ci, (k, t0, ct) in enumerate(calls):
                    g = gpool.tile([128, CT * D], BF16, tag="g")
                    if "gather" not in ablate:
                        nc.gpsimd.dma_gather(
                            g[:, :ct * D].rearrange("p (t d) -> p t d", d=D),
                            tab[k * BUCK:(k + 1) * BUCK, :],
                            idxs[:, t0 * 8:(t0 + ct) * 8],
                            ct * 128, ct * 128, D,
                            single_packet=False,
                            queue_num=ci % 4,
                        )
                    # batched one-hot build for the call's tiles, (r,t) layout
                    # buffer layout is fixed-stride CT: col = r*CT + t
                    ohb = opool.tile([128, 128 * CT], BF16, tag="ohb")
                    rlb = rl[:, t0:t0 + ct].unsqueeze(1).to_broadcast(
                        (128, 128, ct))
                    cwb = cw[:, t0:t0 + ct].unsqueeze(1).to_broadcast(
                        (128, 128, ct))
                    io3 = io[:].rearrange("p (r t) -> p r t", t=CT)[:, :, :ct]
                    oh3 = ohb[:].rearrange("p (r t) -> p r t", t=CT)[:, :, :ct]
                    if "oh" not in ablate:
                        nc.vector.tensor_tensor(
                            out=oh3, in0=io3, in1=rlb,
                            op=mybir.AluOpType.is_equal)
                        if "w" not in ablate:
                            nc.vector.tensor_tensor(
                                out=oh3, in0=oh3, in1=cwb,
                                op=mybir.AluOpType.mult)
                    oh_t = ohb[:].rearrange("p (r t) -> p t r", t=CT)
                    if "mm" not in ablate:
                        for ti in range(ct):
                            t = t0 + ti
                            b = int(tile_blk[t])
                            st = first_tile[b] == t
                            sp = last_tile[b] == t
                            if st:
                                accs[b] = apool.tile([128, 128], F32,
                                                     name=f"accb{b}",
                                                     tag=f"acc{b % BG}")
                            nc.tensor.matmul(
                                out=accs[b][:],
                                lhsT=g[:, ti * D:(ti + 1) * D],
                                rhs=oh_t[:, ti],
                                start=bool(st), stop=bool(sp))
                        if ci in grp_last_call:
                            for b in sorted(accs.keys()):
                                tail(b)
    nc.compile()
    return nc


# ---------------------------------------------------------------------------
# full kernel
# ---------------------------------------------------------------------------

_CACHE = {}


def _prep_inputs(inputs):
    x = np.asarray(inputs["x"], np.float32)
    row = np.asarray(inputs["edge_index"][0], np.int64)
    col = np.asarray(inputs["edge_index"][1], np.int64)
    cv = np.asarray(inputs["C_values"], np.float32)
    l1, l2, T1, sched = build_layouts(row, col, cv)
    x_pad = np.zeros((NPAD, D), np.float32)
    x_pad[:N] = x
    return x_pad, l1, l2, T1, sched


def _launch1_inmaps(x_pad, l1, T1):
    iof = np.tile(np.arange(T1, dtype=np.float32), (128, 1))
    return [
        {"x_sl": x_pad[c * RPC:(c + 1) * RPC], "wvt": l1[c]["wvs"],
         "ptt": l1[c]["ptT"], "ttt": l1[c]["ttB"], "iop": IOTA_P,
         "iof": iof, "isht": ISHIFT, "idnt": IDENT}
        for c in range(NCORES)
    ]


def _iot_host():
    # io[e, r*CT + t] = r
    return np.tile(np.repeat(np.arange(128), CT).astype(BF)[None, :],
                   (128, 1))


def _launch2_inmaps(xs_full, dis, disT, l2, W1, b1):
    w1t = np.ascontiguousarray(np.asarray(W1, np.float32).T).astype(BF)
    iot = _iot_host()
    idn = IDENT.astype(BF)
    return [
        {"tab": xs_full, "rlt": l2[c]["rlt"], "cwt": l2[c]["cwt"],
         "idxt": l2[c]["idxw"], "iot": iot, "dis_t": dis[c],
         "disT_t": disT[c], "w1t": w1t, "idnt": idn,
         "b1t": np.asarray(b1, np.float32).reshape(H, 1)}
        for c in range(NCORES)
    ]


def _launch3_inmaps(hs_full, dis, l2, W2, b2):
    w2t = np.zeros((H, C_PAD), np.float32)
    w2t[:, :C_OUT] = np.asarray(W2, np.float32).T
    b2b = np.zeros((128, C_PAD), np.float32)
    b2b[:, :C_OUT] = np.asarray(b2, np.float32)
    iot = _iot_host()
    return [
        {"tab": hs_full, "rlt": l2[c]["rlt"], "cwt": l2[c]["cwt"],
         "idxt": l2[c]["idxw"], "iot": iot, "dis_t": dis[c],
         "w2t": w2t.astype(BF), "b2t": b2b}
        for c in range(NCORES)
    ]


def kernel(x, edge_index, C_values, W1, b1, W2, b2):
    from concourse.bass_utils import run_bass_kernel_spmd

    inputs = {"x": x, "edge_index": edge_index, "C_values": C_values}
    x_pad, l1, l2, T1, sched = _prep_inputs(inputs)

    key = (T1, sched["NT"], tuple(sched["tile_blk"][::97]))
    if key not in _CACHE:
        _CACHE.clear()
        _CACHE[key] = (build_launch1(T1), build_spmm(sched, 1),
                       build_spmm(sched, 2))
    nc1, nc2, nc3 = _CACHE[key]

    cores = list(range(NCORES))
    r1 = run_bass_kernel_spmd(nc1, _launch1_inmaps(x_pad, l1, T1),
                              core_ids=cores, trace=False)
    xs_full = np.concatenate([r1.results[c]["xs_sl"] for c in cores], axis=0)
    dis = [r1.results[c]["dis_sl"] for c in cores]
    disT = [r1.results[c]["disT_sl"] for c in cores]

    r2 = run_bass_kernel_spmd(
        nc2, _launch2_inmaps(xs_full, dis, disT, l2, W1, b1),
        core_ids=cores, trace=False)
    hs_full = np.concatenate([r2.results[c]["out_sl"] for c in cores], axis=0)

    r3 = run_bass_kernel_spmd(
        nc3, _launch3_inmaps(hs_full, dis, l2, W2, b2),
        core_ids=cores, trace=False)
    out = np.concatenate([r3.results[c]["out_sl"] for c in cores], axis=0)
    return np.ascontiguousarray(out[:N, :C_OUT])


def bench_hw_times(inputs, bench_launch, R=51):
    """Per-launch device time via reps-diff (work/bench_v2.py)."""
    from concourse.bass_utils import run_bass_kernel_spmd

    x_pad, l1, l2, T1, sched = _prep_inputs(inputs)
    cores = list(range(NCORES))
    hw = {}

    in1 = _launch1_inmaps(x_pad, l1, T1)
    nc1a = build_launch1(T1)
    nc1b = build_launch1(T1, reps=R)
    hw["launch1"] = bench_launch(nc1a, nc1b, R, in1, cores)
    r1 = run_bass_kernel_spmd(nc1a, in1, core_ids=cores, trace=False)
    xs_full = np.concatenate([r1.results[c]["xs_sl"] for c in cores], axis=0)
    dis = [r1.results[c]["dis_sl"] for c in cores]
    disT = [r1.results[c]["disT_sl"] for c in cores]

    in2 = _launch2_inmaps(xs_full, dis, disT, l2, inputs["W1"], inputs["b1"])
    nc2a = build_spmm(sched, 1)
    nc2b = build_spmm(sched, 1, reps=R)
    hw["launch2"] = bench_launch(nc2a, nc2b, R, in2, cores)
    r2 = run_bass_kernel_spmd(nc2a, in2, core_ids=cores, trace=False)
    hs_full = np.concatenate([r2.results[c]["out_sl"] for c in cores], axis=0)

    in3 = _launch3_inmaps(hs_full, dis, l2, inputs["W2"], inputs["b2"])
    nc3a = build_spmm(sched, 2)
    nc3b = build_spmm(sched, 2, reps=R)
    hw["launch3"] = bench_launch(nc3a, nc3b, R, in3, cores)
    return hw



# revision 22
# speedup vs baseline: 48369.1057x; 48369.1057x over previous
"""Trainium2 Bass kernel v4 for the 2-layer GCN (nn_CGNN_70566312673786).

8 NeuronCores, SPMD, 3 launches, host concat between launches.

launch1: deg via exact scan-prefix extraction, dis=deg^-1/2, xs=dis*x
         (bf16 table).  x load / xs store batched 14 blocks per DMA; the
         per-block row broadcast for boundary extraction runs on PE as an
         outer product (ones^T @ row), never on gpsimd.
launch2: L1 spmm (one-hot matmul scatter, batched one-hot builds) + W1+relu
         -> hs table (bf16, dis-prescaled for L2's source side).
launch3: L2 spmm + W2 + b2 -> output.

Edge schedule (uniform across cores => one NEFF per launch):
  per (block, bucket): TBB[b,k] = max over cores of ceil(edges/128) tiles.
  Buckets are UNEVEN {32768,32768,32768,2048} (int16 idx limit is 32767);
  this kills the ceil resonance at cnt ~= 512 = 4*128 that even 25088-wide
  buckets hit (NT 1958 -> 1863).  Tiles grouped by (block-group of BG
  blocks, bucket) into segments; gather calls chunk each segment (<= CT=64
  tiles per call); pads use idx 0 with weight 0.  One-hot builds are
  batched: one tensor_tensor(is_equal) + one tensor_tensor(mult) over
  [128, 128*K] in an (r, t) layout whose innermost dim is packed (DVE 2x
  eligible), with stride-0 broadcast rl/cw.  Accumulation: acc[feat, row]
  += g_tile.T @ oh_tile per tile (PSUM bank per block slot, 7 slots).

HW pathologies found on TRN2 (via reps-diff wall timing; each costs 10-50x
if triggered):
  - gpsimd.partition_broadcast in the spmm tail serializes the Q7 cluster
    against SWDGE gather descriptor generation (launch2 3.7ms -> 0.36ms
    when removed).  Nothing on gpsimd in the spmm launches except the
    gathers themselves.
  - Group-batched output stores through a rearranged DRAM AP
    ("(c p) d -> p c d") are ~50x slower than per-block [128, OW] stores
    in the spmm launches (batch_store=False).  The same pattern is fine in
    launch1.
  - Tail math: dis[row] applied post-transpose as per-partition scalars;
    b1 ships host-pre-broadcast [128, H]; relu folds into a DVE
    tensor_scalar (mult dis, max 0) since dis >= 0 commutes with relu
    (tail_act=False keeps the whole tail off the ACT engine).

Per-core HW times (reps-diff, R=1601): launch1 ~30us, launch2 ~158us,
launch3 ~172us; total ~360us vs the 4788us v2 baseline.
"""

import numpy as np
import ml_dtypes

import concourse.bacc as bacc
import concourse.mybir as mybir
import concourse.tile as tile

N = 100000
E = 1600000
D = 128
H = 128
C_OUT = 40
C_PAD = 64

F32 = mybir.dt.float32
BF16 = mybir.dt.bfloat16
I16 = mybir.dt.int16

NCORES = 8
RPC = 12544
NPAD = NCORES * RPC
NBLK = RPC // 128            # 98
NBUCK = 4
BUCK = 32768                 # uneven buckets: 3 x 32768 + 1 x 2048
BUCK_SH = 15                 # (kills ceil resonance at cnt ~= 512 = 4*128)
BG = 7                       # blocks per PSUM group (7 acc banks + 1 tail)
NGRP = (NBLK + BG - 1) // BG
CT = 64                      # max tiles per dma_gather call / oh batch
SENT = 200.0

BF = ml_dtypes.bfloat16

IDENT = np.eye(128, dtype=np.float32)
ISHIFT = np.eye(128, k=-1, dtype=np.float32).T.copy()   # [p,r]=1 iff p==r-1
IOTA_P = np.tile(np.arange(128, dtype=np.float32)[:, None], (1, 128))


def _wrap_idx(flat):
    n = flat.shape[0]
    w = flat.reshape(n // 16, 16).T
    return np.tile(w, (8, 1))


# ---------------------------------------------------------------------------
# host-side schedule (index prep only)
# ---------------------------------------------------------------------------

def build_layouts(row, col, cv):
    row = np.asarray(row, np.int64)
    col = np.asarray(col, np.int64)
    cv = np.asarray(cv, np.float32)
    core_of = row // RPC

    per_core = []
    for c in range(NCORES):
        m = core_of == c
        per_core.append(((row[m] - c * RPC).astype(np.int64), col[m],
                         cv[m].astype(np.float32)))

    # ---- launch1 arrays ----
    T1 = 1
    cnts = []
    for r, _, _ in per_core:
        cb = np.bincount(r >> 7, minlength=NBLK)
        cnts.append(cb)
        T1 = max(T1, int(np.ceil(cb.max() / 128)))
    l1 = []
    for c in range(NCORES):
        r, _, w = per_core[c]
        o = np.argsort(r, kind="stable")
        rs, ws = r[o], w[o]
        blk = rs >> 7
        start_b = np.zeros(NBLK + 1, np.int64)
        np.cumsum(cnts[c], out=start_b[1:])
        e_in_blk = np.arange(len(rs)) - start_b[blk]
        wv = np.zeros((NBLK, 128 * T1), np.float32)
        wv[blk, e_in_blk] = ws
        # chunk layout [b, p, t]: flat position e -> (p=e//T1, t=e%T1)
        wvs = np.ascontiguousarray(
            wv.reshape(NBLK, 128, T1).transpose(1, 0, 2).reshape(
                128, NBLK * T1))
        ends = np.searchsorted(rs, np.arange(RPC), side="right")
        e_r = ends - start_b[np.arange(RPC) >> 7]
        pos = e_r - 1
        pt = np.where(pos >= 0, pos // T1, SENT).astype(np.float32)
        tt = np.where(pos >= 0, pos % T1, 0).astype(np.float32)
        l1.append({
            "wvs": wvs,
            "ptT": np.ascontiguousarray(pt.reshape(1, NBLK * 128)),
            "ttB": np.ascontiguousarray(tt.reshape(NBLK, 128).T),
        })

    # ---- launch2/3 uniform tile structure ----
    nbk_cnt = np.zeros((NCORES, NBLK, NBUCK), np.int64)
    for c in range(NCORES):
        r, cc, _ = per_core[c]
        np.add.at(nbk_cnt[c], ((r >> 7), np.minimum(cc >> BUCK_SH,
                                                    NBUCK - 1)), 1)
    tbb = np.maximum(1, (nbk_cnt.max(axis=0) + 127) // 128)   # [NBLK, NBUCK]

    # tile order: for grp, for bucket, for block in grp: tbb[b,k] tiles
    tile_blk = []
    tile_bkt = []
    seg_of_call = []
    calls = []          # (bucket, tile0, ct)
    grp_last_call = []  # index of last call per grp
    for g in range(NGRP):
        blks = range(g * BG, min((g + 1) * BG, NBLK))
        for k in range(NBUCK):
            seg_t0 = len(tile_blk)
            for b in blks:
                tile_blk += [b] * int(tbb[b, k])
                tile_bkt += [k] * int(tbb[b, k])
            nt = len(tile_blk) - seg_t0
            t = seg_t0
            while t < seg_t0 + nt:
                ct = min(CT, seg_t0 + nt - t)
                calls.append((k, t, ct))
                t += ct
        grp_last_call.append(len(calls) - 1)
    NT = len(tile_blk)
    tile_blk = np.asarray(tile_blk)
    tile_bkt = np.asarray(tile_bkt)

    # tile offset of each (b,k) run
    run_off = np.zeros((NBLK, NBUCK), np.int64)
    pos = 0
    for g in range(NGRP):
        blks = range(g * BG, min((g + 1) * BG, NBLK))
        for k in range(NBUCK):
            for b in blks:
                run_off[b, k] = pos
                pos += int(tbb[b, k])

    # start/stop tile per block
    first_tile = np.full(NBLK, -1, np.int64)
    last_tile = np.full(NBLK, -1, np.int64)
    for t in range(NT):
        b = tile_blk[t]
        if first_tile[b] < 0:
            first_tile[b] = t
        last_tile[b] = t

    # per-core data arrays
    l2 = []
    for c in range(NCORES):
        r, cc, w = per_core[c]
        bk = np.minimum(cc >> BUCK_SH, NBUCK - 1)
        o = np.lexsort((r, bk, (r >> 7)))
        r2, c2, w2, k2 = r[o], cc[o], w[o], bk[o]
        b2 = r2 >> 7
        # position within (b,k) run
        key = b2 * NBUCK + k2
        order = np.lexsort((np.arange(len(r2)), key))
        # edges already grouped by (b,k) under lexsort(r, bk, blk)? group by
        # key directly:
        cnt = np.bincount(key, minlength=NBLK * NBUCK)
        st = np.zeros(NBLK * NBUCK + 1, np.int64)
        np.cumsum(cnt, out=st[1:])
        # within-run position for each edge (in 'order' ordering)
        posn = np.empty(len(r2), np.int64)
        posn[order] = np.arange(len(r2)) - st[key[order]]
        slot = run_off[b2, k2] * 128 + posn
        rlt = np.zeros((128, NT), np.float32)
        cwt = np.zeros((128, NT), np.float32)
        idxf = np.zeros(NT * 128, np.int64)
        tile_of = slot >> 7
        rlt[slot & 127, tile_of] = (r2 & 127).astype(np.float32)
        cwt[slot & 127, tile_of] = w2
        idxf[slot] = c2 - k2 * BUCK
        l2.append({"rlt": rlt.astype(BF), "cwt": cwt.astype(BF),
                   "idxw": _wrap_idx(idxf.astype(np.int16))})

    sched = {"NT": NT, "tile_blk": tile_blk, "tile_bkt": tile_bkt,
             "calls": calls, "grp_last_call": grp_last_call,
             "first_tile": first_tile, "last_tile": last_tile}
    return l1, l2, T1, sched


# ---------------------------------------------------------------------------
# launch 1
# ---------------------------------------------------------------------------

def build_launch1(T1, reps=1):
    import contextlib
    CH = 14                      # blocks per batched x load/store (98 = 7*14)
    nc = bacc.Bacc("TRN2", target_bir_lowering=False)
    x_sl = nc.dram_tensor("x_sl", [RPC, D], F32, kind="ExternalInput")
    wvt = nc.dram_tensor("wvt", [128, NBLK * T1], F32, kind="ExternalInput")
    ptt = nc.dram_tensor("ptt", [1, NBLK * 128], F32, kind="ExternalInput")
    ttt = nc.dram_tensor("ttt", [128, NBLK], F32, kind="ExternalInput")
    iop = nc.dram_tensor("iop", [128, 128], F32, kind="ExternalInput")
    iof = nc.dram_tensor("iof", [128, T1], F32, kind="ExternalInput")
    isht = nc.dram_tensor("isht", [128, 128], F32, kind="ExternalInput")
    idnt = nc.dram_tensor("idnt", [128, 128], F32, kind="ExternalInput")
    xs_sl = nc.dram_tensor("xs_sl", [RPC, D], BF16, kind="ExternalOutput")
    dis_sl = nc.dram_tensor("dis_sl", [128, NBLK], F32, kind="ExternalOutput")
    disT_sl = nc.dram_tensor("disT_sl", [NBLK, 128], F32,
                             kind="ExternalOutput")

    with tile.TileContext(nc) as tc:
        with tc.tile_pool(name="const", bufs=1) as cpool, \
             tc.tile_pool(name="sc", bufs=1) as scpool, \
             tc.tile_pool(name="work", bufs=4) as wpool, \
             tc.tile_pool(name="xio", bufs=3) as xpool, \
             tc.tile_pool(name="small", bufs=4) as spool, \
             tc.tile_pool(name="psA", bufs=2, space="PSUM") as ppool, \
             tc.tile_pool(name="psB", bufs=1, space="PSUM") as ppool2, \
             tc.tile_pool(name="psC", bufs=1, space="PSUM") as ppool3:
            wv = cpool.tile([128, NBLK * T1], F32)
            ttS = cpool.tile([128, NBLK], F32)
            ioP = cpool.tile([128, 128], F32)
            ioF = cpool.tile([128, T1], F32)
            ish = cpool.tile([128, 128], F32)
            idn = cpool.tile([128, 128], F32)
            ptS = cpool.tile([1, NBLK * 128], F32)
            ones = cpool.tile([1, 128], F32)
            zT1 = cpool.tile([128, T1], F32)
            znb = cpool.tile([NBLK, 128], F32)
            E_all = cpool.tile([128, NBLK], F32)
            Sall = scpool.tile([128, NBLK * T1], F32)
            totP = cpool.tile([128, NBLK], F32)
            nc.sync.dma_start(out=wv[:], in_=wvt[:, :])
            nc.sync.dma_start(out=ttS[:], in_=ttt[:, :])
            nc.sync.dma_start(out=ioP[:], in_=iop[:, :])
            nc.sync.dma_start(out=ioF[:], in_=iof[:, :])
            nc.sync.dma_start(out=ish[:], in_=isht[:, :])
            nc.sync.dma_start(out=idn[:], in_=idnt[:, :])
            nc.sync.dma_start(out=ptS[:], in_=ptt[:, :])
            nc.vector.memset(zT1[:], 0.0)
            nc.vector.memset(znb[:], 0.0)
            nc.vector.memset(ones[:], 1.0)

            rep = tc.For_i(0, reps, 1) if reps > 1 else contextlib.nullcontext()
            with rep:
                # A: per-block chunk scans; totals
                for b in range(NBLK):
                    nc.vector.tensor_tensor_scan(
                        out=Sall[:, b * T1:(b + 1) * T1],
                        data0=wv[:, b * T1:(b + 1) * T1],
                        data1=zT1[:], initial=0.0,
                        op0=mybir.AluOpType.add, op1=mybir.AluOpType.add)
                    nc.vector.tensor_copy(
                        out=totP[:, b:b + 1],
                        in_=Sall[:, b * T1 + T1 - 1:b * T1 + T1])
                # carry: transpose -> per-block scan along chunks -> back
                tot_ps = ppool2.tile([NBLK, 128], F32, tag="tps")
                nc.tensor.transpose(out=tot_ps[:], in_=totP[:],
                                    identity=idn[:])
                totT = spool.tile([NBLK, 128], F32, tag="totT")
                nc.vector.tensor_copy(out=totT[:], in_=tot_ps[:])
                incl = spool.tile([NBLK, 128], F32, tag="incl")
                nc.vector.tensor_tensor_scan(
                    out=incl[:], data0=totT[:], data1=znb[:], initial=0.0,
                    op0=mybir.AluOpType.add, op1=mybir.AluOpType.add)
                excl = spool.tile([NBLK, 128], F32, tag="excl")
                nc.vector.tensor_tensor(
                    out=excl[:], in0=incl[:], in1=totT[:],
                    op=mybir.AluOpType.subtract)
                car_ps = ppool2.tile([128, NBLK], F32, tag="cps")
                nc.tensor.transpose(out=car_ps[:], in_=excl[:],
                                    identity=idn[:NBLK, :NBLK])
                carry = spool.tile([128, NBLK], F32, tag="carry")
                nc.vector.tensor_copy(out=carry[:], in_=car_ps[:])

                # B: per-block boundary extraction (row bcast via PE outer
                # product; no gpsimd)
                for b in range(NBLK):
                    pref = wpool.tile([128, T1], F32, tag="pref")
                    nc.vector.tensor_scalar(
                        out=pref[:], in0=Sall[:, b * T1:(b + 1) * T1],
                        scalar1=carry[:, b:b + 1], scalar2=None,
                        op0=mybir.AluOpType.add)
                    ptB_ps = ppool3.tile([128, 128], F32, tag="ptB")
                    nc.tensor.matmul(out=ptB_ps[:], lhsT=ones[:],
                                     rhs=ptS[:, b * 128:(b + 1) * 128],
                                     start=True, stop=True)
                    E1 = wpool.tile([128, 128], F32, tag="E1")
                    nc.vector.tensor_tensor(
                        out=E1[:], in0=ioP[:], in1=ptB_ps[:],
                        op=mybir.AluOpType.is_equal)
                    M_ps = ppool.tile([128, T1], F32, tag="M")
                    nc.tensor.matmul(out=M_ps[:], lhsT=E1[:], rhs=pref[:],
                                     start=True, stop=True)
                    sel = wpool.tile([128, T1], F32, tag="sel")
                    nc.vector.tensor_scalar(
                        out=sel[:], in0=ioF[:], scalar1=ttS[:, b:b + 1],
                        scalar2=None, op0=mybir.AluOpType.is_equal)
                    Msel = wpool.tile([128, T1], F32, tag="Msel")
                    nc.vector.tensor_tensor(
                        out=Msel[:], in0=M_ps[:], in1=sel[:],
                        op=mybir.AluOpType.mult)
                    nc.vector.tensor_reduce(
                        out=E_all[:, b:b + 1], in_=Msel[:],
                        axis=mybir.AxisListType.X, op=mybir.AluOpType.add)

                # C: deg = E - shift(E); dis = rsqrt guard
                Ep_ps = ppool.tile([128, NBLK], F32, tag="Ep")
                nc.tensor.matmul(out=Ep_ps[:], lhsT=ish[:], rhs=E_all[:],
                                 start=True, stop=True)
                deg = spool.tile([128, NBLK], F32, tag="deg")
                nc.vector.tensor_tensor(
                    out=deg[:], in0=E_all[:], in1=Ep_ps[:],
                    op=mybir.AluOpType.subtract)
                z = spool.tile([128, NBLK], F32, tag="z")
                nc.vector.tensor_scalar(
                    out=z[:], in0=deg[:], scalar1=0.0, scalar2=None,
                    op0=mybir.AluOpType.is_le)
                nc.vector.tensor_tensor(
                    out=deg[:], in0=deg[:], in1=z[:], op=mybir.AluOpType.add)
                sq = spool.tile([128, NBLK], F32, tag="sq")
                nc.scalar.sqrt(out=sq[:], in_=deg[:])
                rec = spool.tile([128, NBLK], F32, tag="rec")
                nc.vector.reciprocal(out=rec[:], in_=sq[:])
                nc.vector.tensor_scalar(
                    out=z[:], in0=z[:], scalar1=-1.0, scalar2=1.0,
                    op0=mybir.AluOpType.mult, op1=mybir.AluOpType.add)
                dis = spool.tile([128, NBLK], F32, tag="dis")
                nc.vector.tensor_tensor(
                    out=dis[:], in0=rec[:], in1=z[:], op=mybir.AluOpType.mult)
                nc.sync.dma_start(out=dis_sl[:, :], in_=dis[:])
                dT_ps = ppool2.tile([NBLK, 128], F32, tag="dT")
                nc.tensor.transpose(out=dT_ps[:], in_=dis[:], identity=idn[:])
                dT = spool.tile([NBLK, 128], F32, tag="dTs")
                nc.vector.tensor_copy(out=dT[:], in_=dT_ps[:])
                nc.sync.dma_start(out=disT_sl[:, :], in_=dT[:])

                # D: xs = dis * x -> bf16, CH blocks per DMA
                for c0 in range(0, NBLK, CH):
                    nb = min(CH, NBLK - c0)
                    xt = xpool.tile([128, CH * D], F32, tag="xt")
                    nc.sync.dma_start(
                        out=xt[:, :nb * D].rearrange("p (c d) -> p c d", d=D),
                        in_=x_sl[c0 * 128:(c0 + nb) * 128, :].rearrange(
                            "(c p) d -> p c d", p=128))
                    xst = xpool.tile([128, CH * D], BF16, tag="xst")
                    for j in range(nb):
                        nc.vector.tensor_scalar(
                            out=xst[:, j * D:(j + 1) * D],
                            in0=xt[:, j * D:(j + 1) * D],
                            scalar1=dis[:, c0 + j:c0 + j + 1],
                            scalar2=None, op0=mybir.AluOpType.mult)
                    nc.sync.dma_start(
                        out=xs_sl[c0 * 128:(c0 + nb) * 128, :].rearrange(
                            "(c p) d -> p c d", p=128),
                        in_=xst[:, :nb * D].rearrange("p (c d) -> p c d", d=D))
    nc.compile()
    return nc


# ---------------------------------------------------------------------------
# launches 2/3
# ---------------------------------------------------------------------------

def build_spmm(sched, layer, reps=1, ablate=(), batch_store=False,
               tail_act=False):
    import contextlib
    NT = sched["NT"]
    calls = sched["calls"]
    tile_blk = sched["tile_blk"]
    first_tile = sched["first_tile"]
    last_tile = sched["last_tile"]
    grp_last_call = set(sched["grp_last_call"])

    nc = bacc.Bacc("TRN2", target_bir_lowering=False, num_swdge_queues=4)
    tab = nc.dram_tensor("tab", [NPAD, D], BF16, kind="ExternalInput")
    rlt = nc.dram_tensor("rlt", [128, NT], BF16, kind="ExternalInput")
    cwt = nc.dram_tensor("cwt", [128, NT], BF16, kind="ExternalInput")
    idxt = nc.dram_tensor("idxt", [128, NT * 8], I16, kind="ExternalInput")
    iot = nc.dram_tensor("iot", [128, 128 * CT], BF16, kind="ExternalInput")
    dis_t = nc.dram_tensor("dis_t", [128, NBLK], F32, kind="ExternalInput")
    if layer == 1:
        w1t = nc.dram_tensor("w1t", [D, H], BF16, kind="ExternalInput")
        b1t = nc.dram_tensor("b1t", [128, H], F32, kind="ExternalInput")
        idnt = nc.dram_tensor("idnt", [128, 128], BF16,
                              kind="ExternalInput")
        out_sl = nc.dram_tensor("out_sl", [RPC, D], BF16,
                                kind="ExternalOutput")
        OW = D
    else:
        w2t = nc.dram_tensor("w2t", [H, C_PAD], BF16, kind="ExternalInput")
        b2t = nc.dram_tensor("b2t", [128, C_PAD], F32, kind="ExternalInput")
        out_sl = nc.dram_tensor("out_sl", [RPC, C_PAD], F32,
                                kind="ExternalOutput")
        OW = C_PAD
    ODT = BF16 if layer == 1 else F32

    with tile.TileContext(nc) as tc:
        with tc.tile_pool(name="const", bufs=1) as cpool, \
             tc.tile_pool(name="gat", bufs=3) as gpool, \
             tc.tile_pool(name="oh", bufs=3) as opool, \
             tc.tile_pool(name="tail", bufs=3) as tpool, \
             tc.tile_pool(name="stg", bufs=2) as stpool, \
             tc.tile_pool(name="acc", bufs=1, space="PSUM") as apool, \
             tc.tile_pool(name="pt", bufs=1, space="PSUM") as ppool2:
            rl = cpool.tile([128, NT], BF16)
            cw = cpool.tile([128, NT], BF16)
            idxs = cpool.tile([128, NT * 8], I16)
            io = cpool.tile([128, 128 * CT], BF16)
            dis = cpool.tile([128, NBLK], F32)
            nc.sync.dma_start(out=rl[:], in_=rlt[:, :])
            nc.sync.dma_start(out=cw[:], in_=cwt[:, :])
            nc.sync.dma_start(out=idxs[:], in_=idxt[:, :])
            nc.sync.dma_start(out=io[:], in_=iot[:, :])
            nc.sync.dma_start(out=dis[:], in_=dis_t[:, :])
            if layer == 1:
                w1s = cpool.tile([D, H], BF16)
                b1s = cpool.tile([128, H], F32)
                idn = cpool.tile([128, 128], BF16)
                nc.sync.dma_start(out=w1s[:], in_=w1t[:, :])
                nc.sync.dma_start(out=b1s[:], in_=b1t[:, :])
                nc.sync.dma_start(out=idn[:], in_=idnt[:, :])
            else:
                w2s = cpool.tile([H, C_PAD], BF16)
                b2s = cpool.tile([128, C_PAD], F32)
                nc.sync.dma_start(out=w2s[:], in_=w2t[:, :])
                nc.sync.dma_start(out=b2s[:], in_=b2t[:, :])

            accs = {}

            def tail(b, stage, j):
                # writes result for block b into stage[:, j*OW:(j+1)*OW]
                acc = accs.pop(b)
                if layer == 1:
                    # acc = [feat, row]; h_row = relu(dis_r*(W1@s)_r + b1);
                    # table entry = dis_r * h_row = relu(dis_r * t2) since
                    # dis_r >= 0 commutes with relu.
                    s_sb = tpool.tile([128, 128], BF16, tag="s_sb")
                    nc.vector.tensor_copy(out=s_sb[:], in_=acc[:])
                    hT_ps = ppool2.tile([H, 128], F32, tag="hT")
                    nc.tensor.matmul(out=hT_ps[:], lhsT=w1s[:], rhs=s_sb[:],
                                     start=True, stop=True)
                    ht_sb = tpool.tile([H, 128], BF16, tag="ht_sb")
                    nc.vector.tensor_copy(out=ht_sb[:], in_=hT_ps[:])
                    hh_ps = ppool2.tile([128, H], BF16, tag="hT")
                    nc.tensor.transpose(out=hh_ps[:], in_=ht_sb[:],
                                        identity=idn[:])
                    t2 = tpool.tile([128, H], F32, tag="t2")
                    nc.vector.tensor_scalar(
                        out=t2[:], in0=hh_ps[:], scalar1=dis[:, b:b + 1],
                        scalar2=None, op0=mybir.AluOpType.mult)
                    nc.vector.tensor_tensor(
                        out=t2[:], in0=t2[:], in1=b1s[:],
                        op=mybir.AluOpType.add)
                    hs_sb = tpool.tile([128, H], BF16, tag="hs_sb")
                    if tail_act:
                        nc.scalar.activation(
                            out=hs_sb[:], in_=t2[:],
                            func=mybir.ActivationFunctionType.Relu,
                            bias=0.0, scale=dis[:, b:b + 1])
                    else:
                        nc.vector.tensor_scalar(
                            out=hs_sb[:], in0=t2[:], scalar1=dis[:, b:b + 1],
                            scalar2=0.0, op0=mybir.AluOpType.mult,
                            op1=mybir.AluOpType.max)
                    nc.sync.dma_start(
                        out=out_sl[b * 128:(b + 1) * 128, :],
                        in_=hs_sb[:])
                else:
                    s_sb = tpool.tile([128, 128], BF16, tag="s_sb")
                    nc.vector.tensor_copy(out=s_sb[:], in_=acc[:])
                    o_ps = ppool2.tile([128, C_PAD], F32, tag="hT")
                    nc.tensor.matmul(out=o_ps[:], lhsT=s_sb[:], rhs=w2s[:],
                                     start=True, stop=True)
                    o_sb = tpool.tile([128, C_PAD], F32, tag="o_sb")
                    nc.vector.tensor_scalar(
                        out=o_sb[:], in0=o_ps[:], scalar1=dis[:, b:b + 1],
                        scalar2=None, op0=mybir.AluOpType.mult)
                    if stage is None:
                        nc.vector.tensor_tensor(
                            out=o_sb[:], in0=o_sb[:],
                            in1=b2s[:], op=mybir.AluOpType.add)
                        nc.sync.dma_start(
                            out=out_sl[b * 128:(b + 1) * 128, :],
                            in_=o_sb[:])
                    else:
                        nc.vector.tensor_tensor(
                            out=stage[:, j * OW:(j + 1) * OW], in0=o_sb[:],
                            in1=b2s[:], op=mybir.AluOpType.add)

            rep = tc.For_i(0, reps, 1) if reps > 1 else contextlib.nullcontext()
            with rep:
                for ci, (k, t0, ct) in enumerate(calls):
                    g = gpool.tile([128, CT * D], BF16, tag="g")
                    if "gather" not in ablate:
                        nc.gpsimd.dma_gather(
                            g[:, :ct * D].rearrange("p (t d) -> p t d", d=D),
                            tab[k * BUCK:min((k + 1) * BUCK, NPAD), :],
                            idxs[:, t0 * 8:(t0 + ct) * 8],
                            ct * 128, ct * 128, D,
                            single_packet=False,
                            queue_num=ci % 4,
                        )
                    # batched one-hot build for the call's tiles, (r,t) layout
                    # buffer layout is fixed-stride CT: col = r*CT + t
                    ohb = opool.tile([128, 128 * CT], BF16, tag="ohb")
                    rlb = rl[:, t0:t0 + ct].unsqueeze(1).to_broadcast(
                        (128, 128, ct))
                    cwb = cw[:, t0:t0 + ct].unsqueeze(1).to_broadcast(
                        (128, 128, ct))
                    io3 = io[:].rearrange("p (r t) -> p r t", t=CT)[:, :, :ct]
                    oh3 = ohb[:].rearrange("p (r t) -> p r t", t=CT)[:, :, :ct]
                    if "oh" not in ablate:
                        nc.vector.tensor_tensor(
                            out=oh3, in0=io3, in1=rlb,
                            op=mybir.AluOpType.is_equal)
                        if "w" not in ablate:
                            nc.vector.tensor_tensor(
                                out=oh3, in0=oh3, in1=cwb,
                                op=mybir.AluOpType.mult)
                    oh_t = ohb[:].rearrange("p (r t) -> p t r", t=CT)
                    if "mm" not in ablate:
                        for ti in range(ct):
                            t = t0 + ti
                            b = int(tile_blk[t])
                            st = first_tile[b] == t
                            sp = last_tile[b] == t
                            if st:
                                accs[b] = apool.tile([128, 128], F32,
                                                     name=f"accb{b}",
                                                     tag=f"acc{b % BG}")
                            nc.tensor.matmul(
                                out=accs[b][:],
                                lhsT=g[:, ti * D:(ti + 1) * D],
                                rhs=oh_t[:, ti],
                                start=bool(st), stop=bool(sp))
                        if ci in grp_last_call:
                            blks = sorted(accs.keys())
                            b0 = blks[0]
                            ng = len(blks)
                            if batch_store:
                                stage = stpool.tile([128, BG * OW], ODT,
                                                    tag="stage")
                            else:
                                stage = None
                            for j, b in enumerate(blks):
                                tail(b, stage, j)
                            if batch_store:
                                nc.sync.dma_start(
                                    out=out_sl[b0 * 128:(b0 + ng) * 128,
                                               :].rearrange(
                                        "(c p) d -> p c d", p=128),
                                    in_=stage[:, :ng * OW].rearrange(
                                        "p (c d) -> p c d", d=OW))
    nc.compile()
    return nc


# ---------------------------------------------------------------------------
# full kernel
# ---------------------------------------------------------------------------

_CACHE = {}


def _prep_inputs(inputs):
    x = np.asarray(inputs["x"], np.float32)
    row = np.asarray(inputs["edge_index"][0], np.int64)
    col = np.asarray(inputs["edge_index"][1], np.int64)
    cv = np.asarray(inputs["C_values"], np.float32)
    l1, l2, T1, sched = build_layouts(row, col, cv)
    x_pad = np.zeros((NPAD, D), np.float32)
    x_pad[:N] = x
    return x_pad, l1, l2, T1, sched


def _launch1_inmaps(x_pad, l1, T1):
    iof = np.tile(np.arange(T1, dtype=np.float32), (128, 1))
    return [
        {"x_sl": x_pad[c * RPC:(c + 1) * RPC], "wvt": l1[c]["wvs"],
         "ptt": l1[c]["ptT"], "ttt": l1[c]["ttB"], "iop": IOTA_P,
         "iof": iof, "isht": ISHIFT, "idnt": IDENT}
        for c in range(NCORES)
    ]


def _iot_host():
    # io[e, r*CT + t] = r
    return np.tile(np.repeat(np.arange(128), CT).astype(BF)[None, :],
                   (128, 1))


def _launch2_inmaps(xs_full, dis, l2, W1, b1):
    w1t = np.ascontiguousarray(np.asarray(W1, np.float32).T).astype(BF)
    iot = _iot_host()
    idn = IDENT.astype(BF)
    b1b = np.tile(np.asarray(b1, np.float32).reshape(1, H), (128, 1))
    return [
        {"tab": xs_full, "rlt": l2[c]["rlt"], "cwt": l2[c]["cwt"],
         "idxt": l2[c]["idxw"], "iot": iot, "dis_t": dis[c],
         "w1t": w1t, "idnt": idn, "b1t": b1b}
        for c in range(NCORES)
    ]


def _launch3_inmaps(hs_full, dis, l2, W2, b2):
    w2t = np.zeros((H, C_PAD), np.float32)
    w2t[:, :C_OUT] = np.asarray(W2, np.float32).T
    b2b = np.zeros((128, C_PAD), np.float32)
    b2b[:, :C_OUT] = np.asarray(b2, np.float32)
    iot = _iot_host()
    return [
        {"tab": hs_full, "rlt": l2[c]["rlt"], "cwt": l2[c]["cwt"],
         "idxt": l2[c]["idxw"], "iot": iot, "dis_t": dis[c],
         "w2t": w2t.astype(BF), "b2t": b2b}
        for c in range(NCORES)
    ]


def kernel(x, edge_index, C_values, W1, b1, W2, b2):
    from concourse.bass_utils import run_bass_kernel_spmd

    inputs = {"x": x, "edge_index": edge_index, "C_values": C_values}
    x_pad, l1, l2, T1, sched = _prep_inputs(inputs)

    key = (T1, sched["NT"], tuple(sched["tile_blk"][::97]))
    if key not in _CACHE:
        _CACHE.clear()
        _CACHE[key] = (build_launch1(T1), build_spmm(sched, 1),
                       build_spmm(sched, 2))
    nc1, nc2, nc3 = _CACHE[key]

    cores = list(range(NCORES))
    r1 = run_bass_kernel_spmd(nc1, _launch1_inmaps(x_pad, l1, T1),
                              core_ids=cores, trace=False)
    xs_full = np.concatenate([r1.results[c]["xs_sl"] for c in cores], axis=0)
    dis = [r1.results[c]["dis_sl"] for c in cores]

    r2 = run_bass_kernel_spmd(
        nc2, _launch2_inmaps(xs_full, dis, l2, W1, b1),
        core_ids=cores, trace=False)
    hs_full = np.concatenate([r2.results[c]["out_sl"] for c in cores], axis=0)

    r3 = run_bass_kernel_spmd(
        nc3, _launch3_inmaps(hs_full, dis, l2, W2, b2),
        core_ids=cores, trace=False)
    out = np.concatenate([r3.results[c]["out_sl"] for c in cores], axis=0)
    return np.ascontiguousarray(out[:N, :C_OUT])


def bench_hw_times(inputs, bench_launch, R=51):
    """Per-launch device time via reps-diff (work/bench_v2.py)."""
    from concourse.bass_utils import run_bass_kernel_spmd

    x_pad, l1, l2, T1, sched = _prep_inputs(inputs)
    cores = list(range(NCORES))
    hw = {}

    in1 = _launch1_inmaps(x_pad, l1, T1)
    nc1a = build_launch1(T1)
    nc1b = build_launch1(T1, reps=R)
    hw["launch1"] = bench_launch(nc1a, nc1b, R, in1, cores)
    r1 = run_bass_kernel_spmd(nc1a, in1, core_ids=cores, trace=False)
    xs_full = np.concatenate([r1.results[c]["xs_sl"] for c in cores], axis=0)
    dis = [r1.results[c]["dis_sl"] for c in cores]

    in2 = _launch2_inmaps(xs_full, dis, l2, inputs["W1"], inputs["b1"])
    nc2a = build_spmm(sched, 1)
    nc2b = build_spmm(sched, 1, reps=R)
    hw["launch2"] = bench_launch(nc2a, nc2b, R, in2, cores)
    r2 = run_bass_kernel_spmd(nc2a, in2, core_ids=cores, trace=False)
    hs_full = np.concatenate([r2.results[c]["out_sl"] for c in cores], axis=0)

    in3 = _launch3_inmaps(hs_full, dis, l2, inputs["W2"], inputs["b2"])
    nc3a = build_spmm(sched, 2)
    nc3b = build_spmm(sched, 2, reps=R)
    hw["launch3"] = bench_launch(nc3a, nc3b, R, in3, cores)
    return hw



# revision 26
# speedup vs baseline: 54757.4781x; 1.1321x over previous
"""Trainium2 Bass kernel v4 for the 2-layer GCN (nn_CGNN_70566312673786).

8 NeuronCores, SPMD, 3 launches, host concat between launches.

launch1: deg via exact scan-prefix extraction, dis=deg^-1/2, xs=dis*x
         (bf16 table).  x load / xs store batched 14 blocks per DMA; the
         per-block row broadcast for boundary extraction runs on PE as an
         outer product (ones^T @ row), never on gpsimd.
launch2: L1 spmm (one-hot matmul scatter, batched one-hot builds) + W1+relu
         -> hs table (bf16, dis-prescaled for L2's source side).
launch3: L2 spmm + W2 + b2 -> output.

Edge schedule (uniform across cores => one NEFF per launch):
  per (block, bucket): TBB[b,k] = max over cores of ceil(edges/128) tiles.
  Buckets are UNEVEN {32768,32768,32768,2048} (int16 idx limit is 32767);
  this kills the ceil resonance at cnt ~= 512 = 4*128 that even 25088-wide
  buckets hit (NT 1958 -> 1863).  Tiles grouped by (block-group of BG
  blocks, bucket) into segments; gather calls chunk each segment (<= CT=64
  tiles per call); pads use idx 0 with weight 0.  One-hot builds are
  batched: one tensor_tensor(is_equal) + one tensor_tensor(mult) over
  [128, 128*K] in an (r, t) layout whose innermost dim is packed (DVE 2x
  eligible), with stride-0 broadcast rl/cw.  Accumulation: acc[feat, row]
  += g_tile.T @ oh_tile per tile (PSUM bank per block slot, 7 slots).

HW pathologies found on TRN2 (via reps-diff wall timing; each costs 10-50x
if triggered):
  - gpsimd.partition_broadcast in the spmm tail serializes the Q7 cluster
    against SWDGE gather descriptor generation (launch2 3.7ms -> 0.36ms
    when removed).  Nothing on gpsimd in the spmm launches except the
    gathers themselves.
  - Group-batched output stores through a rearranged DRAM AP
    ("(c p) d -> p c d") are ~50x slower than per-block [128, OW] stores
    in the spmm launches (batch_store=False).  The same pattern is fine in
    launch1.
  - Tail math: dis[row] applied post-transpose as per-partition scalars;
    b1 ships host-pre-broadcast [128, H]; relu folds into a DVE
    tensor_scalar (mult dis, max 0) since dis >= 0 commutes with relu
    (tail_act=False keeps the whole tail off the ACT engine).

Gather/one-hot pools are 4-deep in launch3 (3-deep in launch2, whose
extra consts leave no SBUF headroom) -- pipeline depth was worth ~20%.

Per-core HW times (reps-diff, R=1601): launch1 ~30us, launch2 ~155us,
launch3 ~133us; total ~318us vs the 4788us v2 baseline (~15x).
"""

import numpy as np
import ml_dtypes

import concourse.bacc as bacc
import concourse.mybir as mybir
import concourse.tile as tile

N = 100000
E = 1600000
D = 128
H = 128
C_OUT = 40
C_PAD = 64

F32 = mybir.dt.float32
BF16 = mybir.dt.bfloat16
I16 = mybir.dt.int16

NCORES = 8
RPC = 12544
NPAD = NCORES * RPC
NBLK = RPC // 128            # 98
NBUCK = 4
BUCK = 32768                 # uneven buckets: 3 x 32768 + 1 x 2048
BUCK_SH = 15                 # (kills ceil resonance at cnt ~= 512 = 4*128)
BG = 7                       # blocks per PSUM group (7 acc banks + 1 tail)
NGRP = (NBLK + BG - 1) // BG
CT = 64                      # max tiles per dma_gather call / oh batch
SENT = 200.0

BF = ml_dtypes.bfloat16

IDENT = np.eye(128, dtype=np.float32)
ISHIFT = np.eye(128, k=-1, dtype=np.float32).T.copy()   # [p,r]=1 iff p==r-1
IOTA_P = np.tile(np.arange(128, dtype=np.float32)[:, None], (1, 128))


def _wrap_idx(flat):
    n = flat.shape[0]
    w = flat.reshape(n // 16, 16).T
    return np.tile(w, (8, 1))


# ---------------------------------------------------------------------------
# host-side schedule (index prep only)
# ---------------------------------------------------------------------------

def build_layouts(row, col, cv):
    row = np.asarray(row, np.int64)
    col = np.asarray(col, np.int64)
    cv = np.asarray(cv, np.float32)
    core_of = row // RPC

    per_core = []
    for c in range(NCORES):
        m = core_of == c
        per_core.append(((row[m] - c * RPC).astype(np.int64), col[m],
                         cv[m].astype(np.float32)))

    # ---- launch1 arrays ----
    T1 = 1
    cnts = []
    for r, _, _ in per_core:
        cb = np.bincount(r >> 7, minlength=NBLK)
        cnts.append(cb)
        T1 = max(T1, int(np.ceil(cb.max() / 128)))
    l1 = []
    for c in range(NCORES):
        r, _, w = per_core[c]
        o = np.argsort(r, kind="stable")
        rs, ws = r[o], w[o]
        blk = rs >> 7
        start_b = np.zeros(NBLK + 1, np.int64)
        np.cumsum(cnts[c], out=start_b[1:])
        e_in_blk = np.arange(len(rs)) - start_b[blk]
        wv = np.zeros((NBLK, 128 * T1), np.float32)
        wv[blk, e_in_blk] = ws
        # chunk layout [b, p, t]: flat position e -> (p=e//T1, t=e%T1)
        wvs = np.ascontiguousarray(
            wv.reshape(NBLK, 128, T1).transpose(1, 0, 2).reshape(
                128, NBLK * T1))
        ends = np.searchsorted(rs, np.arange(RPC), side="right")
        e_r = ends - start_b[np.arange(RPC) >> 7]
        pos = e_r - 1
        pt = np.where(pos >= 0, pos // T1, SENT).astype(np.float32)
        tt = np.where(pos >= 0, pos % T1, 0).astype(np.float32)
        l1.append({
            "wvs": wvs,
            "ptT": np.ascontiguousarray(pt.reshape(1, NBLK * 128)),
            "ttB": np.ascontiguousarray(tt.reshape(NBLK, 128).T),
        })

    # ---- launch2/3 uniform tile structure ----
    nbk_cnt = np.zeros((NCORES, NBLK, NBUCK), np.int64)
    for c in range(NCORES):
        r, cc, _ = per_core[c]
        np.add.at(nbk_cnt[c], ((r >> 7), np.minimum(cc >> BUCK_SH,
                                                    NBUCK - 1)), 1)
    tbb = np.maximum(1, (nbk_cnt.max(axis=0) + 127) // 128)   # [NBLK, NBUCK]

    # tile order: for grp, for bucket, for block in grp: tbb[b,k] tiles
    tile_blk = []
    tile_bkt = []
    seg_of_call = []
    calls = []          # (bucket, tile0, ct)
    grp_last_call = []  # index of last call per grp
    for g in range(NGRP):
        blks = range(g * BG, min((g + 1) * BG, NBLK))
        for k in range(NBUCK):
            seg_t0 = len(tile_blk)
            for b in blks:
                tile_blk += [b] * int(tbb[b, k])
                tile_bkt += [k] * int(tbb[b, k])
            nt = len(tile_blk) - seg_t0
            t = seg_t0
            while t < seg_t0 + nt:
                ct = min(CT, seg_t0 + nt - t)
                calls.append((k, t, ct))
                t += ct
        grp_last_call.append(len(calls) - 1)
    NT = len(tile_blk)
    tile_blk = np.asarray(tile_blk)
    tile_bkt = np.asarray(tile_bkt)

    # tile offset of each (b,k) run
    run_off = np.zeros((NBLK, NBUCK), np.int64)
    pos = 0
    for g in range(NGRP):
        blks = range(g * BG, min((g + 1) * BG, NBLK))
        for k in range(NBUCK):
            for b in blks:
                run_off[b, k] = pos
                pos += int(tbb[b, k])

    # start/stop tile per block
    first_tile = np.full(NBLK, -1, np.int64)
    last_tile = np.full(NBLK, -1, np.int64)
    for t in range(NT):
        b = tile_blk[t]
        if first_tile[b] < 0:
            first_tile[b] = t
        last_tile[b] = t

    # per-core data arrays
    l2 = []
    for c in range(NCORES):
        r, cc, w = per_core[c]
        bk = np.minimum(cc >> BUCK_SH, NBUCK - 1)
        o = np.lexsort((r, bk, (r >> 7)))
        r2, c2, w2, k2 = r[o], cc[o], w[o], bk[o]
        b2 = r2 >> 7
        # position within (b,k) run
        key = b2 * NBUCK + k2
        order = np.lexsort((np.arange(len(r2)), key))
        # edges already grouped by (b,k) under lexsort(r, bk, blk)? group by
        # key directly:
        cnt = np.bincount(key, minlength=NBLK * NBUCK)
        st = np.zeros(NBLK * NBUCK + 1, np.int64)
        np.cumsum(cnt, out=st[1:])
        # within-run position for each edge (in 'order' ordering)
        posn = np.empty(len(r2), np.int64)
        posn[order] = np.arange(len(r2)) - st[key[order]]
        slot = run_off[b2, k2] * 128 + posn
        rlt = np.zeros((128, NT), np.float32)
        cwt = np.zeros((128, NT), np.float32)
        idxf = np.zeros(NT * 128, np.int64)
        tile_of = slot >> 7
        rlt[slot & 127, tile_of] = (r2 & 127).astype(np.float32)
        cwt[slot & 127, tile_of] = w2
        idxf[slot] = c2 - k2 * BUCK
        l2.append({"rlt": rlt.astype(BF), "cwt": cwt.astype(BF),
                   "idxw": _wrap_idx(idxf.astype(np.int16))})

    sched = {"NT": NT, "tile_blk": tile_blk, "tile_bkt": tile_bkt,
             "calls": calls, "grp_last_call": grp_last_call,
             "first_tile": first_tile, "last_tile": last_tile}
    return l1, l2, T1, sched


# ---------------------------------------------------------------------------
# launch 1
# ---------------------------------------------------------------------------

def build_launch1(T1, reps=1):
    import contextlib
    CH = 14                      # blocks per batched x load/store (98 = 7*14)
    nc = bacc.Bacc("TRN2", target_bir_lowering=False)
    x_sl = nc.dram_tensor("x_sl", [RPC, D], F32, kind="ExternalInput")
    wvt = nc.dram_tensor("wvt", [128, NBLK * T1], F32, kind="ExternalInput")
    ptt = nc.dram_tensor("ptt", [1, NBLK * 128], F32, kind="ExternalInput")
    ttt = nc.dram_tensor("ttt", [128, NBLK], F32, kind="ExternalInput")
    iop = nc.dram_tensor("iop", [128, 128], F32, kind="ExternalInput")
    iof = nc.dram_tensor("iof", [128, T1], F32, kind="ExternalInput")
    isht = nc.dram_tensor("isht", [128, 128], F32, kind="ExternalInput")
    idnt = nc.dram_tensor("idnt", [128, 128], F32, kind="ExternalInput")
    xs_sl = nc.dram_tensor("xs_sl", [RPC, D], BF16, kind="ExternalOutput")
    dis_sl = nc.dram_tensor("dis_sl", [128, NBLK], F32, kind="ExternalOutput")
    disT_sl = nc.dram_tensor("disT_sl", [NBLK, 128], F32,
                             kind="ExternalOutput")

    with tile.TileContext(nc) as tc:
        with tc.tile_pool(name="const", bufs=1) as cpool, \
             tc.tile_pool(name="sc", bufs=1) as scpool, \
             tc.tile_pool(name="work", bufs=4) as wpool, \
             tc.tile_pool(name="xio", bufs=3) as xpool, \
             tc.tile_pool(name="small", bufs=4) as spool, \
             tc.tile_pool(name="psA", bufs=2, space="PSUM") as ppool, \
             tc.tile_pool(name="psB", bufs=1, space="PSUM") as ppool2, \
             tc.tile_pool(name="psC", bufs=1, space="PSUM") as ppool3:
            wv = cpool.tile([128, NBLK * T1], F32)
            ttS = cpool.tile([128, NBLK], F32)
            ioP = cpool.tile([128, 128], F32)
            ioF = cpool.tile([128, T1], F32)
            ish = cpool.tile([128, 128], F32)
            idn = cpool.tile([128, 128], F32)
            ptS = cpool.tile([1, NBLK * 128], F32)
            ones = cpool.tile([1, 128], F32)
            zT1 = cpool.tile([128, T1], F32)
            znb = cpool.tile([NBLK, 128], F32)
            E_all = cpool.tile([128, NBLK], F32)
            Sall = scpool.tile([128, NBLK * T1], F32)
            totP = cpool.tile([128, NBLK], F32)
            nc.sync.dma_start(out=wv[:], in_=wvt[:, :])
            nc.sync.dma_start(out=ttS[:], in_=ttt[:, :])
            nc.sync.dma_start(out=ioP[:], in_=iop[:, :])
            nc.sync.dma_start(out=ioF[:], in_=iof[:, :])
            nc.sync.dma_start(out=ish[:], in_=isht[:, :])
            nc.sync.dma_start(out=idn[:], in_=idnt[:, :])
            nc.sync.dma_start(out=ptS[:], in_=ptt[:, :])
            nc.vector.memset(zT1[:], 0.0)
            nc.vector.memset(znb[:], 0.0)
            nc.vector.memset(ones[:], 1.0)

            rep = tc.For_i(0, reps, 1) if reps > 1 else contextlib.nullcontext()
            with rep:
                # A: per-block chunk scans; totals
                for b in range(NBLK):
                    nc.vector.tensor_tensor_scan(
                        out=Sall[:, b * T1:(b + 1) * T1],
                        data0=wv[:, b * T1:(b + 1) * T1],
                        data1=zT1[:], initial=0.0,
                        op0=mybir.AluOpType.add, op1=mybir.AluOpType.add)
                    nc.vector.tensor_copy(
                        out=totP[:, b:b + 1],
                        in_=Sall[:, b * T1 + T1 - 1:b * T1 + T1])
                # carry: transpose -> per-block scan along chunks -> back
                tot_ps = ppool2.tile([NBLK, 128], F32, tag="tps")
                nc.tensor.transpose(out=tot_ps[:], in_=totP[:],
                                    identity=idn[:])
                totT = spool.tile([NBLK, 128], F32, tag="totT")
                nc.vector.tensor_copy(out=totT[:], in_=tot_ps[:])
                incl = spool.tile([NBLK, 128], F32, tag="incl")
                nc.vector.tensor_tensor_scan(
                    out=incl[:], data0=totT[:], data1=znb[:], initial=0.0,
                    op0=mybir.AluOpType.add, op1=mybir.AluOpType.add)
                excl = spool.tile([NBLK, 128], F32, tag="excl")
                nc.vector.tensor_tensor(
                    out=excl[:], in0=incl[:], in1=totT[:],
                    op=mybir.AluOpType.subtract)
                car_ps = ppool2.tile([128, NBLK], F32, tag="cps")
                nc.tensor.transpose(out=car_ps[:], in_=excl[:],
                                    identity=idn[:NBLK, :NBLK])
                carry = spool.tile([128, NBLK], F32, tag="carry")
                nc.vector.tensor_copy(out=carry[:], in_=car_ps[:])

                # B: per-block boundary extraction (row bcast via PE outer
                # product; no gpsimd)
                for b in range(NBLK):
                    pref = wpool.tile([128, T1], F32, tag="pref")
                    nc.vector.tensor_scalar(
                        out=pref[:], in0=Sall[:, b * T1:(b + 1) * T1],
                        scalar1=carry[:, b:b + 1], scalar2=None,
                        op0=mybir.AluOpType.add)
                    ptB_ps = ppool3.tile([128, 128], F32, tag="ptB")
                    nc.tensor.matmul(out=ptB_ps[:], lhsT=ones[:],
                                     rhs=ptS[:, b * 128:(b + 1) * 128],
                                     start=True, stop=True)
                    E1 = wpool.tile([128, 128], F32, tag="E1")
                    nc.vector.tensor_tensor(
                        out=E1[:], in0=ioP[:], in1=ptB_ps[:],
                        op=mybir.AluOpType.is_equal)
                    M_ps = ppool.tile([128, T1], F32, tag="M")
                    nc.tensor.matmul(out=M_ps[:], lhsT=E1[:], rhs=pref[:],
                                     start=True, stop=True)
                    sel = wpool.tile([128, T1], F32, tag="sel")
                    nc.vector.tensor_scalar(
                        out=sel[:], in0=ioF[:], scalar1=ttS[:, b:b + 1],
                        scalar2=None, op0=mybir.AluOpType.is_equal)
                    Msel = wpool.tile([128, T1], F32, tag="Msel")
                    nc.vector.tensor_tensor(
                        out=Msel[:], in0=M_ps[:], in1=sel[:],
                        op=mybir.AluOpType.mult)
                    nc.vector.tensor_reduce(
                        out=E_all[:, b:b + 1], in_=Msel[:],
                        axis=mybir.AxisListType.X, op=mybir.AluOpType.add)

                # C: deg = E - shift(E); dis = rsqrt guard
                Ep_ps = ppool.tile([128, NBLK], F32, tag="Ep")
                nc.tensor.matmul(out=Ep_ps[:], lhsT=ish[:], rhs=E_all[:],
                                 start=True, stop=True)
                deg = spool.tile([128, NBLK], F32, tag="deg")
                nc.vector.tensor_tensor(
                    out=deg[:], in0=E_all[:], in1=Ep_ps[:],
                    op=mybir.AluOpType.subtract)
                z = spool.tile([128, NBLK], F32, tag="z")
                nc.vector.tensor_scalar(
                    out=z[:], in0=deg[:], scalar1=0.0, scalar2=None,
                    op0=mybir.AluOpType.is_le)
                nc.vector.tensor_tensor(
                    out=deg[:], in0=deg[:], in1=z[:], op=mybir.AluOpType.add)
                sq = spool.tile([128, NBLK], F32, tag="sq")
                nc.scalar.sqrt(out=sq[:], in_=deg[:])
                rec = spool.tile([128, NBLK], F32, tag="rec")
                nc.vector.reciprocal(out=rec[:], in_=sq[:])
                nc.vector.tensor_scalar(
                    out=z[:], in0=z[:], scalar1=-1.0, scalar2=1.0,
                    op0=mybir.AluOpType.mult, op1=mybir.AluOpType.add)
                dis = spool.tile([128, NBLK], F32, tag="dis")
                nc.vector.tensor_tensor(
                    out=dis[:], in0=rec[:], in1=z[:], op=mybir.AluOpType.mult)
                nc.sync.dma_start(out=dis_sl[:, :], in_=dis[:])
                dT_ps = ppool2.tile([NBLK, 128], F32, tag="dT")
                nc.tensor.transpose(out=dT_ps[:], in_=dis[:], identity=idn[:])
                dT = spool.tile([NBLK, 128], F32, tag="dTs")
                nc.vector.tensor_copy(out=dT[:], in_=dT_ps[:])
                nc.sync.dma_start(out=disT_sl[:, :], in_=dT[:])

                # D: xs = dis * x -> bf16, CH blocks per DMA
                for c0 in range(0, NBLK, CH):
                    nb = min(CH, NBLK - c0)
                    xt = xpool.tile([128, CH * D], F32, tag="xt")
                    nc.sync.dma_start(
                        out=xt[:, :nb * D].rearrange("p (c d) -> p c d", d=D),
                        in_=x_sl[c0 * 128:(c0 + nb) * 128, :].rearrange(
                            "(c p) d -> p c d", p=128))
                    xst = xpool.tile([128, CH * D], BF16, tag="xst")
                    for j in range(nb):
                        nc.vector.tensor_scalar(
                            out=xst[:, j * D:(j + 1) * D],
                            in0=xt[:, j * D:(j + 1) * D],
                            scalar1=dis[:, c0 + j:c0 + j + 1],
                            scalar2=None, op0=mybir.AluOpType.mult)
                    nc.sync.dma_start(
                        out=xs_sl[c0 * 128:(c0 + nb) * 128, :].rearrange(
                            "(c p) d -> p c d", p=128),
                        in_=xst[:, :nb * D].rearrange("p (c d) -> p c d", d=D))
    nc.compile()
    return nc


# ---------------------------------------------------------------------------
# launches 2/3
# ---------------------------------------------------------------------------

def build_spmm(sched, layer, reps=1, ablate=(), batch_store=False,
               tail_act=False, act_cast=False):
    import contextlib
    NT = sched["NT"]
    calls = sched["calls"]
    tile_blk = sched["tile_blk"]
    first_tile = sched["first_tile"]
    last_tile = sched["last_tile"]
    grp_last_call = set(sched["grp_last_call"])

    nc = bacc.Bacc("TRN2", target_bir_lowering=False, num_swdge_queues=4)
    tab = nc.dram_tensor("tab", [NPAD, D], BF16, kind="ExternalInput")
    rlt = nc.dram_tensor("rlt", [128, NT], BF16, kind="ExternalInput")
    cwt = nc.dram_tensor("cwt", [128, NT], BF16, kind="ExternalInput")
    idxt = nc.dram_tensor("idxt", [128, NT * 8], I16, kind="ExternalInput")
    iot = nc.dram_tensor("iot", [128, 128 * CT], BF16, kind="ExternalInput")
    dis_t = nc.dram_tensor("dis_t", [128, NBLK], F32, kind="ExternalInput")
    if layer == 1:
        w1t = nc.dram_tensor("w1t", [D, H], BF16, kind="ExternalInput")
        b1t = nc.dram_tensor("b1t", [128, H], F32, kind="ExternalInput")
        idnt = nc.dram_tensor("idnt", [128, 128], BF16,
                              kind="ExternalInput")
        out_sl = nc.dram_tensor("out_sl", [RPC, D], BF16,
                                kind="ExternalOutput")
        OW = D
    else:
        w2t = nc.dram_tensor("w2t", [H, C_PAD], BF16, kind="ExternalInput")
        b2t = nc.dram_tensor("b2t", [128, C_PAD], F32, kind="ExternalInput")
        out_sl = nc.dram_tensor("out_sl", [RPC, C_PAD], F32,
                                kind="ExternalOutput")
        OW = C_PAD
    ODT = BF16 if layer == 1 else F32
    PBUFS = 3 if layer == 1 else 4   # layer1's extra consts leave no SBUF
                                     # headroom for 4-deep gather pipeline

    with tile.TileContext(nc) as tc:
        with tc.tile_pool(name="const", bufs=1) as cpool, \
             tc.tile_pool(name="gat", bufs=PBUFS) as gpool, \
             tc.tile_pool(name="oh", bufs=PBUFS) as opool, \
             tc.tile_pool(name="tail", bufs=3) as tpool, \
             tc.tile_pool(name="stg", bufs=2) as stpool, \
             tc.tile_pool(name="acc", bufs=1, space="PSUM") as apool, \
             tc.tile_pool(name="pt", bufs=1, space="PSUM") as ppool2:
            rl = cpool.tile([128, NT], BF16)
            cw = cpool.tile([128, NT], BF16)
            idxs = cpool.tile([128, NT * 8], I16)
            io = cpool.tile([128, 128 * CT], BF16)
            dis = cpool.tile([128, NBLK], F32)
            nc.sync.dma_start(out=rl[:], in_=rlt[:, :])
            nc.sync.dma_start(out=cw[:], in_=cwt[:, :])
            nc.sync.dma_start(out=idxs[:], in_=idxt[:, :])
            nc.sync.dma_start(out=io[:], in_=iot[:, :])
            nc.sync.dma_start(out=dis[:], in_=dis_t[:, :])
            if layer == 1:
                w1s = cpool.tile([D, H], BF16)
                b1s = cpool.tile([128, H], F32)
                idn = cpool.tile([128, 128], BF16)
                nc.sync.dma_start(out=w1s[:], in_=w1t[:, :])
                nc.sync.dma_start(out=b1s[:], in_=b1t[:, :])
                nc.sync.dma_start(out=idn[:], in_=idnt[:, :])
            else:
                w2s = cpool.tile([H, C_PAD], BF16)
                b2s = cpool.tile([128, C_PAD], F32)
                nc.sync.dma_start(out=w2s[:], in_=w2t[:, :])
                nc.sync.dma_start(out=b2s[:], in_=b2t[:, :])

            accs = {}

            def tail(b, stage, j):
                # writes result for block b into stage[:, j*OW:(j+1)*OW]
                acc = accs.pop(b)
                if layer == 1:
                    # acc = [feat, row]; h_row = relu(dis_r*(W1@s)_r + b1);
                    # table entry = dis_r * h_row = relu(dis_r * t2) since
                    # dis_r >= 0 commutes with relu.
                    s_sb = tpool.tile([128, 128], BF16, tag="s_sb")
                    if act_cast:
                        nc.scalar.activation(
                            out=s_sb[:], in_=acc[:],
                            func=mybir.ActivationFunctionType.Copy)
                    else:
                        nc.vector.tensor_copy(out=s_sb[:], in_=acc[:])
                    hT_ps = ppool2.tile([H, 128], F32, tag="hT")
                    nc.tensor.matmul(out=hT_ps[:], lhsT=w1s[:], rhs=s_sb[:],
                                     start=True, stop=True)
                    ht_sb = tpool.tile([H, 128], BF16, tag="ht_sb")
                    if act_cast:
                        nc.scalar.activation(
                            out=ht_sb[:], in_=hT_ps[:],
                            func=mybir.ActivationFunctionType.Copy)
                    else:
                        nc.vector.tensor_copy(out=ht_sb[:], in_=hT_ps[:])
                    hh_ps = ppool2.tile([128, H], BF16, tag="hT")
                    nc.tensor.transpose(out=hh_ps[:], in_=ht_sb[:],
                                        identity=idn[:])
                    t2 = tpool.tile([128, H], F32, tag="t2")
                    nc.vector.tensor_scalar(
                        out=t2[:], in0=hh_ps[:], scalar1=dis[:, b:b + 1],
                        scalar2=None, op0=mybir.AluOpType.mult)
                    nc.vector.tensor_tensor(
                        out=t2[:], in0=t2[:], in1=b1s[:],
                        op=mybir.AluOpType.add)
                    hs_sb = tpool.tile([128, H], BF16, tag="hs_sb")
                    if tail_act:
                        nc.scalar.activation(
                            out=hs_sb[:], in_=t2[:],
                            func=mybir.ActivationFunctionType.Relu,
                            bias=0.0, scale=dis[:, b:b + 1])
                    else:
                        nc.vector.tensor_scalar(
                            out=hs_sb[:], in0=t2[:], scalar1=dis[:, b:b + 1],
                            scalar2=0.0, op0=mybir.AluOpType.mult,
                            op1=mybir.AluOpType.max)
                    nc.sync.dma_start(
                        out=out_sl[b * 128:(b + 1) * 128, :],
                        in_=hs_sb[:])
                else:
                    s_sb = tpool.tile([128, 128], BF16, tag="s_sb")
                    if act_cast:
                        nc.scalar.activation(
                            out=s_sb[:], in_=acc[:],
                            func=mybir.ActivationFunctionType.Copy)
                    else:
                        nc.vector.tensor_copy(out=s_sb[:], in_=acc[:])
                    o_ps = ppool2.tile([128, C_PAD], F32, tag="hT")
                    nc.tensor.matmul(out=o_ps[:], lhsT=s_sb[:], rhs=w2s[:],
                                     start=True, stop=True)
                    o_sb = tpool.tile([128, C_PAD], F32, tag="o_sb")
                    nc.vector.tensor_scalar(
                        out=o_sb[:], in0=o_ps[:], scalar1=dis[:, b:b + 1],
                        scalar2=None, op0=mybir.AluOpType.mult)
                    if stage is None:
                        nc.vector.tensor_tensor(
                            out=o_sb[:], in0=o_sb[:],
                            in1=b2s[:], op=mybir.AluOpType.add)
                        nc.sync.dma_start(
                            out=out_sl[b * 128:(b + 1) * 128, :],
                            in_=o_sb[:])
                    else:
                        nc.vector.tensor_tensor(
                            out=stage[:, j * OW:(j + 1) * OW], in0=o_sb[:],
                            in1=b2s[:], op=mybir.AluOpType.add)

            rep = tc.For_i(0, reps, 1) if reps > 1 else contextlib.nullcontext()
            with rep:
                for ci, (k, t0, ct) in enumerate(calls):
                    g = gpool.tile([128, CT * D], BF16, tag="g")
                    if "gather" not in ablate:
                        nc.gpsimd.dma_gather(
                            g[:, :ct * D].rearrange("p (t d) -> p t d", d=D),
                            tab[k * BUCK:min((k + 1) * BUCK, NPAD), :],
                            idxs[:, t0 * 8:(t0 + ct) * 8],
                            ct * 128, ct * 128, D,
                            single_packet=False,
                            queue_num=ci % 4,
                        )
                    # batched one-hot build for the call's tiles, (r,t) layout
                    # buffer layout is fixed-stride CT: col = r*CT + t
                    ohb = opool.tile([128, 128 * CT], BF16, tag="ohb")
                    rlb = rl[:, t0:t0 + ct].unsqueeze(1).to_broadcast(
                        (128, 128, ct))
                    cwb = cw[:, t0:t0 + ct].unsqueeze(1).to_broadcast(
                        (128, 128, ct))
                    io3 = io[:].rearrange("p (r t) -> p r t", t=CT)[:, :, :ct]
                    oh3 = ohb[:].rearrange("p (r t) -> p r t", t=CT)[:, :, :ct]
                    if "oh" not in ablate:
                        nc.vector.tensor_tensor(
                            out=oh3, in0=io3, in1=rlb,
                            op=mybir.AluOpType.is_equal)
                        if "w" not in ablate:
                            nc.vector.tensor_tensor(
                                out=oh3, in0=oh3, in1=cwb,
                                op=mybir.AluOpType.mult)
                    oh_t = ohb[:].rearrange("p (r t) -> p t r", t=CT)
                    if "mm" not in ablate:
                        for ti in range(ct):
                            t = t0 + ti
                            b = int(tile_blk[t])
                            st = first_tile[b] == t
                            sp = last_tile[b] == t
                            if st:
                                accs[b] = apool.tile([128, 128], F32,
                                                     name=f"accb{b}",
                                                     tag=f"acc{b % BG}")
                            nc.tensor.matmul(
                                out=accs[b][:],
                                lhsT=g[:, ti * D:(ti + 1) * D],
                                rhs=oh_t[:, ti],
                                start=bool(st), stop=bool(sp))
                        if ci in grp_last_call:
                            blks = sorted(accs.keys())
                            b0 = blks[0]
                            ng = len(blks)
                            if batch_store:
                                stage = stpool.tile([128, BG * OW], ODT,
                                                    tag="stage")
                            else:
                                stage = None
                            for j, b in enumerate(blks):
                                tail(b, stage, j)
                            if batch_store:
                                nc.sync.dma_start(
                                    out=out_sl[b0 * 128:(b0 + ng) * 128,
                                               :].rearrange(
                                        "(c p) d -> p c d", p=128),
                                    in_=stage[:, :ng * OW].rearrange(
                                        "p (c d) -> p c d", d=OW))
    nc.compile()
    return nc


# ---------------------------------------------------------------------------
# full kernel
# ---------------------------------------------------------------------------

_CACHE = {}


def _prep_inputs(inputs):
    x = np.asarray(inputs["x"], np.float32)
    row = np.asarray(inputs["edge_index"][0], np.int64)
    col = np.asarray(inputs["edge_index"][1], np.int64)
    cv = np.asarray(inputs["C_values"], np.float32)
    l1, l2, T1, sched = build_layouts(row, col, cv)
    x_pad = np.zeros((NPAD, D), np.float32)
    x_pad[:N] = x
    return x_pad, l1, l2, T1, sched


def _launch1_inmaps(x_pad, l1, T1):
    iof = np.tile(np.arange(T1, dtype=np.float32), (128, 1))
    return [
        {"x_sl": x_pad[c * RPC:(c + 1) * RPC], "wvt": l1[c]["wvs"],
         "ptt": l1[c]["ptT"], "ttt": l1[c]["ttB"], "iop": IOTA_P,
         "iof": iof, "isht": ISHIFT, "idnt": IDENT}
        for c in range(NCORES)
    ]


def _iot_host():
    # io[e, r*CT + t] = r
    return np.tile(np.repeat(np.arange(128), CT).astype(BF)[None, :],
                   (128, 1))


def _launch2_inmaps(xs_full, dis, l2, W1, b1):
    w1t = np.ascontiguousarray(np.asarray(W1, np.float32).T).astype(BF)
    iot = _iot_host()
    idn = IDENT.astype(BF)
    b1b = np.tile(np.asarray(b1, np.float32).reshape(1, H), (128, 1))
    return [
        {"tab": xs_full, "rlt": l2[c]["rlt"], "cwt": l2[c]["cwt"],
         "idxt": l2[c]["idxw"], "iot": iot, "dis_t": dis[c],
         "w1t": w1t, "idnt": idn, "b1t": b1b}
        for c in range(NCORES)
    ]


def _launch3_inmaps(hs_full, dis, l2, W2, b2):
    w2t = np.zeros((H, C_PAD), np.float32)
    w2t[:, :C_OUT] = np.asarray(W2, np.float32).T
    b2b = np.zeros((128, C_PAD), np.float32)
    b2b[:, :C_OUT] = np.asarray(b2, np.float32)
    iot = _iot_host()
    return [
        {"tab": hs_full, "rlt": l2[c]["rlt"], "cwt": l2[c]["cwt"],
         "idxt": l2[c]["idxw"], "iot": iot, "dis_t": dis[c],
         "w2t": w2t.astype(BF), "b2t": b2b}
        for c in range(NCORES)
    ]


def kernel(x, edge_index, C_values, W1, b1, W2, b2):
    from concourse.bass_utils import run_bass_kernel_spmd

    inputs = {"x": x, "edge_index": edge_index, "C_values": C_values}
    x_pad, l1, l2, T1, sched = _prep_inputs(inputs)

    key = (T1, sched["NT"], tuple(sched["tile_blk"][::97]))
    if key not in _CACHE:
        _CACHE.clear()
        _CACHE[key] = (build_launch1(T1), build_spmm(sched, 1),
                       build_spmm(sched, 2))
    nc1, nc2, nc3 = _CACHE[key]

    cores = list(range(NCORES))
    r1 = run_bass_kernel_spmd(nc1, _launch1_inmaps(x_pad, l1, T1),
                              core_ids=cores, trace=False)
    xs_full = np.concatenate([r1.results[c]["xs_sl"] for c in cores], axis=0)
    dis = [r1.results[c]["dis_sl"] for c in cores]

    r2 = run_bass_kernel_spmd(
        nc2, _launch2_inmaps(xs_full, dis, l2, W1, b1),
        core_ids=cores, trace=False)
    hs_full = np.concatenate([r2.results[c]["out_sl"] for c in cores], axis=0)

    r3 = run_bass_kernel_spmd(
        nc3, _launch3_inmaps(hs_full, dis, l2, W2, b2),
        core_ids=cores, trace=False)
    out = np.concatenate([r3.results[c]["out_sl"] for c in cores], axis=0)
    return np.ascontiguousarray(out[:N, :C_OUT])


def bench_hw_times(inputs, bench_launch, R=51):
    """Per-launch device time via reps-diff (work/bench_v2.py)."""
    from concourse.bass_utils import run_bass_kernel_spmd

    x_pad, l1, l2, T1, sched = _prep_inputs(inputs)
    cores = list(range(NCORES))
    hw = {}

    in1 = _launch1_inmaps(x_pad, l1, T1)
    nc1a = build_launch1(T1)
    nc1b = build_launch1(T1, reps=R)
    hw["launch1"] = bench_launch(nc1a, nc1b, R, in1, cores)
    r1 = run_bass_kernel_spmd(nc1a, in1, core_ids=cores, trace=False)
    xs_full = np.concatenate([r1.results[c]["xs_sl"] for c in cores], axis=0)
    dis = [r1.results[c]["dis_sl"] for c in cores]

    in2 = _launch2_inmaps(xs_full, dis, l2, inputs["W1"], inputs["b1"])
    nc2a = build_spmm(sched, 1)
    nc2b = build_spmm(sched, 1, reps=R)
    hw["launch2"] = bench_launch(nc2a, nc2b, R, in2, cores)
    r2 = run_bass_kernel_spmd(nc2a, in2, core_ids=cores, trace=False)
    hs_full = np.concatenate([r2.results[c]["out_sl"] for c in cores], axis=0)

    in3 = _launch3_inmaps(hs_full, dis, l2, inputs["W2"], inputs["b2"])
    nc3a = build_spmm(sched, 2)
    nc3b = build_spmm(sched, 2, reps=R)
    hw["launch3"] = bench_launch(nc3a, nc3b, R, in3, cores)
    return hw

